# revision 11
# baseline (speedup 1.0000x reference)
"""Bass/Tile kernel for nn_BVRNN: GRU-based variational RNN forward on trn2.

The recurrence is strictly sequential in T with batch 64, so the recurrent
loop runs on a single core (SPMD-replicated across all 8; core 0's result is
used). All matmuls are weight-stationary (lhsT = W tile) with activations
kept feature-major [feat_partition, batch_free]. The phi_x MLP and the phi_x
half of enc layer 1 are precomputed for all (b, t) before the loop (the "E"
contribution), stored t-major in DRAM and streamed per 8-step group.

ELU outputs are stored in '+1' form (elu(x)+1 = max(pre+1, exp(min(pre, 0)))),
with the -1 correction folded into downstream biases via column sums.
z = round(sigmoid(u)) is computed as (u >= -b) directly from the preact psum.
Only Exp/Ln ACT functions are used; sigmoid/tanh are emulated with
exp + reciprocal. kld is computed in a vectorized post-pass from staged
enc/prior preactivations.
"""

import numpy as np

import concourse.bass as bass
import concourse.mybir as mybir
import concourse.tile as tile
from concourse import bacc
from concourse.bass import ds, ts
from concourse.bass_utils import run_bass_kernel_spmd
from concourse.masks import make_identity

F32 = mybir.dt.float32
AF = mybir.ActivationFunctionType
ALU = mybir.AluOpType

X_DIM, H_DIM, Z_DIM = 80, 512, 64
B = 64
HC = H_DIM // 128  # 4

_CACHE = {}


def _colsum(W):
    return np.asarray(W, np.float32).sum(axis=0)


def _pc(b, C):
    b = np.asarray(b, np.float32)
    return np.ascontiguousarray(b.reshape(C, 128).T)


def _bc(b, C):
    p = _pc(b, C)
    return np.ascontiguousarray(np.repeat(p[:, :, None], B, axis=2).reshape(128, C * B))


def _prep_inputs(y, mean_mel, std_mel, phi_x_params, phi_z_params, enc_params,
                 prior_params, dec_params, gru_params):
    f32 = lambda x: np.ascontiguousarray(np.asarray(x, np.float32))
    (Wx1, bx1), (Wx2, bx2), (Wx3, bx3) = [(f32(w), f32(b)) for w, b in phi_x_params]
    (Wz1, bz1), (Wz2, bz2), (Wz3, bz3) = [(f32(w), f32(b)) for w, b in phi_z_params]
    (We1, be1), (We2, be2), (We3, be3) = [(f32(w), f32(b)) for w, b in enc_params]
    (Wp1, bp1), (Wp2, bp2), (Wp3, bp3) = [(f32(w), f32(b)) for w, b in prior_params]
    (Wd1, bd1), (Wd2, bd2), (Wd3, bd3), (Wd4, bd4) = [(f32(w), f32(b)) for w, b in dec_params]
    w_ih, w_hh, b_ih, b_hh = [f32(x) for x in gru_params]
    mean = f32(mean_mel)
    std = f32(std_mel)

    Wx1n = Wx1 / std[:, None]
    bx1n = bx1 - (mean / std) @ Wx1

    bx2p = bx2 - _colsum(Wx2)
    bx3p = bx3 - _colsum(Wx3)
    beE = be1 - _colsum(We1[:H_DIM])
    be2p = be2 - _colsum(We2)
    be3p = be3 - _colsum(We3)
    bz2p = bz2 - _colsum(Wz2)
    bz3p = bz3 - _colsum(Wz3)
    bd1p = bd1 - _colsum(Wd1[:H_DIM])
    bd2p = bd2 - _colsum(Wd2)
    bd3p = bd3 - _colsum(Wd3)
    bd4p = bd4 - _colsum(Wd4)
    b_ihp = b_ih - _colsum(w_ih[:H_DIM]) - _colsum(w_ih[H_DIM:])
    bp2p = bp2 - _colsum(Wp2)
    bp3p = bp3 - _colsum(Wp3)

    Wz1e = np.concatenate([Wz1, bz1[None, :]], axis=0)       # [65, 512]
    Wx1e = np.concatenate([Wx1n, bx1n[None, :]], axis=0)     # [81, 512]

    brz = (b_ihp + b_hh)[: 2 * H_DIM]
    bin_ = b_ihp[2 * H_DIM:]
    bhn = b_hh[2 * H_DIM:]

    return {
        "y": f32(y),
        "Wx1n": Wx1n, "bx1n_a": _pc(bx1n, HC), "bx1n_b": _pc(bx1n + 1.0, HC),
        "Wx2": Wx2, "bx2_a": _pc(bx2p, HC), "bx2_b": _pc(bx2p + 1.0, HC),
        "Wx3": Wx3, "bx3_a": _pc(bx3p, HC), "bx3_b": _pc(bx3p + 1.0, HC),
        "We1t": np.ascontiguousarray(We1[:H_DIM]),
        "We1b": np.ascontiguousarray(We1[H_DIM:]),
        "beE": _pc(beE, HC),
        "We2": We2, "be2_a": _pc(be2p, HC), "be2_b": _pc(be2p + 1.0, HC),
        "We3": We3, "be3": np.ascontiguousarray(be3p.reshape(Z_DIM, 1)),
        "th3": np.ascontiguousarray((-be3p).reshape(Z_DIM, 1)),
        "Wz1e": Wz1e,
        "Wz2": Wz2, "bz2_a": _pc(bz2p, HC), "bz2_b": _pc(bz2p + 1.0, HC),
        "Wz3": Wz3, "bz3_a": _pc(bz3p, HC), "bz3_b": _pc(bz3p + 1.0, HC),
        "Wd1": Wd1, "bd1_a": _pc(bd1p, HC), "bd1_b": _pc(bd1p + 1.0, HC),
        "Wd2": Wd2, "bd2_a": _pc(bd2p, HC), "bd2_b": _pc(bd2p + 1.0, HC),
        "Wd3": Wd3, "bd3_a": _pc(bd3p, HC), "bd3_b": _pc(bd3p + 1.0, HC),
        "Wd4": Wd4, "bd4": np.ascontiguousarray(bd4p.reshape(X_DIM, 1)),
        "Wx1e": Wx1e,
        "Wih": w_ih, "Whh": w_hh,
        "brz8": _pc(brz, 8), "bin4": _pc(bin_, HC), "bhn4": _pc(bhn, HC),
        "Wp1": Wp1, "bp1_a": _pc(bp1, HC), "bp1_b": _pc(bp1 + 1.0, HC),
        "Wp2": Wp2, "bp2_a": _pc(bp2p, HC), "bp2_b": _pc(bp2p + 1.0, HC),
        "Wp3": Wp3, "bp3": np.ascontiguousarray(bp3p.reshape(Z_DIM, 1)),
    }


def _load_w(nc, pool, dram, K, M):
    """Load W [K, M] into SBUF tile [128 or K, K//128 or 1, M]."""
    tag = "w_" + dram.name
    if K > 128:
        assert K % 128 == 0
        t = pool.tile([128, K // 128, M], F32, tag=tag)
        nc.sync.dma_start(t[:], dram[:].rearrange("(c p) m -> p c m", p=128))
    else:
        t = pool.tile([K, 1, M], F32, tag=tag)
        nc.sync.dma_start(t[:, 0, :], dram[:])
    return t


def _ld_bias(nc, pool, dram, shape):
    t = pool.tile(list(shape), F32, tag="b_" + dram.name)
    nc.sync.dma_start(t[:], dram[:])
    return t


def _elu_p1(nc, mnpool, epool, psums, out, b_a, b_b, nb, extra=None, prepool=None):
    """out[:, m, :] = elu(psum_m + bias_m [+ extra_m]) + 1.

    psums: list of per-m psum APs [128, nb]. b_a/b_b: [128, C] tiles or None
    (bias already included; uses 0/+1 immediates). extra: [128, C, nb] sbuf AP
    added to the preact (enc1's E contribution).
    """
    C = len(psums)
    mn = mnpool.tile([128, C, nb], F32, tag="mn")
    if extra is not None:
        pre = prepool.tile([128, C, nb], F32, tag="pre")
        for m in range(C):
            nc.vector.scalar_tensor_tensor(pre[:, m, :], psums[m], 0.0,
                                           extra[:, m, :], op0=ALU.add, op1=ALU.add)
        for m in range(C):
            nc.vector.tensor_scalar(mn[:, m, :], pre[:, m, :], 0.0, None, op0=ALU.min)
    elif b_a is not None:
        for m in range(C):
            nc.vector.tensor_scalar(mn[:, m, :], psums[m], b_a[:, m:m + 1], 0.0,
                                    op0=ALU.add, op1=ALU.min)
    else:
        for m in range(C):
            nc.vector.tensor_scalar(mn[:, m, :], psums[m], 0.0, None, op0=ALU.min)
    e = epool.tile([128, C, nb], F32, tag="ee")
    nc.scalar.activation(e[:], mn[:], AF.Exp)
    for m in range(C):
        if extra is not None:
            nc.vector.scalar_tensor_tensor(out[:, m, :], pre[:, m, :], 1.0,
                                           e[:, m, :], op0=ALU.add, op1=ALU.max)
        elif b_b is not None:
            nc.vector.scalar_tensor_tensor(out[:, m, :], psums[m], b_b[:, m:m + 1],
                                           e[:, m, :], op0=ALU.add, op1=ALU.max)
        else:
            nc.vector.scalar_tensor_tensor(out[:, m, :], psums[m], 1.0,
                                           e[:, m, :], op0=ALU.add, op1=ALU.max)


def _build(T):
    assert T % 8 == 0
    NS = T // 8

    nc = bacc.Bacc("TRN2", target_bir_lowering=False, debug=False, num_devices=8)

    d = {}
    def din(name, shape):
        d[name] = nc.dram_tensor(name, list(shape), F32, kind="ExternalInput")
    din("y", (B, T, X_DIM))
    din("Wx1n", (X_DIM, H_DIM)); din("bx1n_a", (128, HC)); din("bx1n_b", (128, HC))
    din("Wx2", (H_DIM, H_DIM)); din("bx2_a", (128, HC)); din("bx2_b", (128, HC))
    din("Wx3", (H_DIM, H_DIM)); din("bx3_a", (128, HC)); din("bx3_b", (128, HC))
    din("We1t", (H_DIM, H_DIM)); din("We1b", (H_DIM, H_DIM)); din("beE", (128, HC))
    din("We2", (H_DIM, H_DIM)); din("be2_a", (128, HC)); din("be2_b", (128, HC))
    din("We3", (H_DIM, Z_DIM)); din("be3", (Z_DIM, 1)); din("th3", (Z_DIM, 1))
    din("Wz1e", (Z_DIM + 1, H_DIM))
    din("Wz2", (H_DIM, H_DIM)); din("bz2_a", (128, HC)); din("bz2_b", (128, HC))
    din("Wz3", (H_DIM, H_DIM)); din("bz3_a", (128, HC)); din("bz3_b", (128, HC))
    din("Wd1", (2 * H_DIM, H_DIM)); din("bd1_a", (128, HC)); din("bd1_b", (128, HC))
    din("Wd2", (H_DIM, H_DIM)); din("bd2_a", (128, HC)); din("bd2_b", (128, HC))
    din("Wd3", (H_DIM, H_DIM)); din("bd3_a", (128, HC)); din("bd3_b", (128, HC))
    din("Wd4", (H_DIM, X_DIM)); din("bd4", (X_DIM, 1))
    din("Wx1e", (X_DIM + 1, H_DIM))
    din("Wih", (2 * H_DIM, 3 * H_DIM)); din("Whh", (H_DIM, 3 * H_DIM))
    din("brz8", (128, 8)); din("bin4", (128, HC)); din("bhn4", (128, HC))
    din("Wp1", (H_DIM, H_DIM)); din("bp1_a", (128, HC)); din("bp1_b", (128, HC))
    din("Wp2", (H_DIM, H_DIM)); din("bp2_a", (128, HC)); din("bp2_b", (128, HC))
    din("Wp3", (H_DIM, Z_DIM)); din("bp3", (Z_DIM, 1))

    DEC = nc.dram_tensor("DEC", [NS, 8, X_DIM, B], F32, kind="ExternalOutput")
    KLD = nc.dram_tensor("KLD", [Z_DIM, 1], F32, kind="ExternalOutput")

    Eflat = nc.dram_tensor("Eflat", [NS, 128, HC * 8 * B], F32)
    ENCP = nc.dram_tensor("ENCP", [Z_DIM, NS, 8 * B], F32)
    PRIP = nc.dram_tensor("PRIP", [Z_DIM, NS, 8 * B], F32)

    with tile.TileContext(nc) as tc:
        _emit_precompute(nc, tc, d, Eflat, T)
        _emit_loop(nc, tc, d, DEC, Eflat, ENCP, PRIP, NS)
        _emit_kld(nc, tc, ENCP, PRIP, KLD, NS)

    nc.compile()
    return nc


def _emit_precompute(nc, tc, d, Eflat, T):
    TCH = 120 if T % 120 == 0 else 8
    NCH = T // TCH
    ROWS = B * TCH
    assert ROWS % 512 == 0
    NSL = ROWS // 512
    TPS = 512 // B  # t's per slice = 8
    with (
        tc.tile_pool(name="pw", bufs=1) as pw,
        tc.tile_pool(name="pa", bufs=2) as pa,
        tc.tile_pool(name="pyt", bufs=2) as pyt,
        tc.tile_pool(name="pps", bufs=6, space="PSUM") as pps,
        tc.tile_pool(name="ppt", bufs=2, space="PSUM") as ppt,
    ):
        ident = pw.tile([128, 128], F32, tag="ident")
        make_identity(nc, ident[:])
        Wx1n_t = _load_w(nc, pw, d["Wx1n"], X_DIM, H_DIM)
        Wx2_t = _load_w(nc, pw, d["Wx2"], H_DIM, H_DIM)
        Wx3_t = _load_w(nc, pw, d["Wx3"], H_DIM, H_DIM)
        We1t_t = _load_w(nc, pw, d["We1t"], H_DIM, H_DIM)
        bb = {}
        for nm in ["bx1n_a", "bx1n_b", "bx2_a", "bx2_b", "bx3_a", "bx3_b", "beE"]:
            bb[nm] = _ld_bias(nc, pw, d[nm], (128, HC))

        for ch in range(NCH):
            yT = pyt.tile([X_DIM, ROWS], F32, tag="yT")
            for b in range(B):
                yblk = pa.tile([TCH, X_DIM], F32, tag="yblk")
                nc.sync.dma_start(yblk[:], d["y"][b, ch * TCH:(ch + 1) * TCH, :])
                tp = ppt.tile([X_DIM, TCH], F32, tag="tp")
                nc.tensor.transpose(tp[:], yblk[:], ident[:TCH, :TCH])
                nc.vector.tensor_copy(yT[:, b * TCH:(b + 1) * TCH], tp[:])

            yTv = yT[:].rearrange("p (b t) -> p t b", b=B)
            for s in range(NSL):
                rhs1 = yTv[:, s * TPS:(s + 1) * TPS, :]

                def layer(W_t, rhs, KC, b_a, b_b, out_tag, single_k=False):
                    psums = []
                    for m in range(HC):
                        pm = pps.tile([128, 512], F32, tag="ps")
                        if single_k:
                            nc.tensor.matmul(pm[:], W_t[:, 0, ts(m, 128)], rhs,
                                             start=True, stop=True)
                        else:
                            for c in range(HC):
                                nc.tensor.matmul(pm[:], W_t[:, c, ts(m, 128)],
                                                 rhs[:, c, :], start=(c == 0),
                                                 stop=(c == HC - 1))
                        psums.append(pm[:])
                    out = pa.tile([128, HC, 512], F32, tag=out_tag)
                    _elu_p1(nc, pa, pa, psums, out, b_a, b_b, 512)
                    return out

                a1 = layer(Wx1n_t, rhs1, 1, bb["bx1n_a"], bb["bx1n_b"], "a1", single_k=True)
                a2 = layer(Wx2_t, a1, HC, bb["bx2_a"], bb["bx2_b"], "a2")
                a3 = layer(Wx3_t, a2, HC, bb["bx3_a"], bb["bx3_b"], "a1")
                eo = pa.tile([128, HC, 512], F32, tag="a2")
                for m in range(HC):
                    pm = pps.tile([128, 512], F32, tag="ps")
                    for c in range(HC):
                        nc.tensor.matmul(pm[:], We1t_t[:, c, ts(m, 128)], a3[:, c, :],
                                         start=(c == 0), stop=(c == HC - 1))
                    nc.vector.tensor_scalar_add(eo[:, m, :], pm[:], bb["beE"][:, m:m + 1])
                gs = (ch * TCH) // TPS + s
                nc.sync.dma_start(
                    Eflat[gs, :, :].rearrange("p (c x) -> p c x", c=HC), eo[:])


def _emit_loop(nc, tc, d, DEC, Eflat, ENCP, PRIP, NS):
    with (
        tc.tile_pool(name="lw", bufs=1) as lw,
        tc.tile_pool(name="st", bufs=1) as st,
        tc.tile_pool(name="io", bufs=1) as io,
        tc.tile_pool(name="e8", bufs=2) as e8p,
        tc.tile_pool(name="ac", bufs=2) as ac,
        tc.tile_pool(name="sm", bufs=1) as sm,
        tc.tile_pool(name="mne", bufs=1) as mne,
        tc.tile_pool(name="lps", bufs=6, space="PSUM") as lps,
        tc.tile_pool(name="gps", bufs=2, space="PSUM") as gps,
    ):
        W = {}
        W["e1b"] = _load_w(nc, lw, d["We1b"], H_DIM, H_DIM)
        W["e2"] = _load_w(nc, lw, d["We2"], H_DIM, H_DIM)
        W["e3"] = _load_w(nc, lw, d["We3"], H_DIM, Z_DIM)
        W["z1"] = _load_w(nc, lw, d["Wz1e"], Z_DIM + 1, H_DIM)
        W["z2"] = _load_w(nc, lw, d["Wz2"], H_DIM, H_DIM)
        W["z3"] = _load_w(nc, lw, d["Wz3"], H_DIM, H_DIM)
        W["d1"] = _load_w(nc, lw, d["Wd1"], 2 * H_DIM, H_DIM)
        W["d2"] = _load_w(nc, lw, d["Wd2"], H_DIM, H_DIM)
        W["d3"] = _load_w(nc, lw, d["Wd3"], H_DIM, H_DIM)
        W["d4"] = _load_w(nc, lw, d["Wd4"], H_DIM, X_DIM)
        W["x1"] = _load_w(nc, lw, d["Wx1e"], X_DIM + 1, H_DIM)
        W["x2"] = _load_w(nc, lw, d["Wx2"], H_DIM, H_DIM)
        W["x3"] = _load_w(nc, lw, d["Wx3"], H_DIM, H_DIM)
        W["ih"] = _load_w(nc, lw, d["Wih"], 2 * H_DIM, 3 * H_DIM)
        W["hh"] = _load_w(nc, lw, d["Whh"], H_DIM, 3 * H_DIM)
        W["p1"] = _load_w(nc, lw, d["Wp1"], H_DIM, H_DIM)
        W["p2"] = _load_w(nc, lw, d["Wp2"], H_DIM, H_DIM)
        W["p3"] = _load_w(nc, lw, d["Wp3"], H_DIM, Z_DIM)

        bias = {}
        for nm in ["bx2", "bx3", "be2", "bz2", "bz3", "bd1", "bd2", "bd3",
                   "bp1", "bp2"]:
            bias[nm + "_a"] = _ld_bias(nc, lw, d[nm + "_a"], (128, HC))
            bias[nm + "_b"] = _ld_bias(nc, lw, d[nm + "_b"], (128, HC))
        bias["be3"] = _ld_bias(nc, lw, d["be3"], (Z_DIM, 1))
        bias["th3"] = _ld_bias(nc, lw, d["th3"], (Z_DIM, 1))
        bias["bd4"] = _ld_bias(nc, lw, d["bd4"], (X_DIM, 1))
        bias["bp3"] = _ld_bias(nc, lw, d["bp3"], (Z_DIM, 1))
        brz_t = _ld_bias(nc, lw, d["brz8"], (128, 8))
        bin_t = _ld_bias(nc, lw, d["bin4"], (128, HC))
        bhn_t = _ld_bias(nc, lw, d["bhn4"], (128, HC))

        h = st.tile([128, HC, B], F32, tag="h")
        nc.vector.memset(h[:], 0.0)

        with tc.For_i(0, NS, 1, hint_engines=tuple(mybir.ALL_ENGINES)) as si:
            Ev = Eflat[ds(si, 1), :, :].rearrange("o p (c t b) -> p c (o t) b",
                                                  c=HC, b=B)
            EP8 = io.tile([128, 8, B], F32, tag="EP8")  # enc preacts [:64], prior [64:]
            for half in range(4):
                E2 = e8p.tile([128, HC, 2, B], F32, tag="E2")
                nc.sync.dma_start(E2[:], Ev[:, :, half * 2:(half + 1) * 2, :])
                for tl2 in range(2):
                    tl = half * 2 + tl2
                    _emit_step(nc, ac, sm, mne, lps, gps, h, E2, EP8, DEC, si, tl,
                               tl2, W, bias, brz_t, bin_t, bhn_t)

            nc.sync.dma_start(
                ENCP[:, ds(si, 1), :].rearrange("p o x -> p (o x)"),
                EP8[:Z_DIM, :, :].rearrange("p t b -> p (t b)"))
            nc.sync.dma_start(
                PRIP[:, ds(si, 1), :].rearrange("p o x -> p (o x)"),
                EP8[Z_DIM:, :, :].rearrange("p t b -> p (t b)"))


def _mm(nc, psum_ap, W_t, rhs, KC, mslice, start=True, stop=True):
    for c in range(KC):
        nc.tensor.matmul(psum_ap, W_t[:, c, mslice], rhs[:, c, :],
                         start=(start and c == 0), stop=(stop and c == KC - 1))


def _emit_step(nc, ac, sm, mne, lps, gps, h, E2, EP8, DEC, si, tl, tl2,
               W, bias, brz_t, bin_t, bhn_t):
    def hc_layer(W_t, rhs, b_a, b_b, out_tag, single_k=False, extra=None,
                 dual=None):
        psums = []
        for m in range(HC):
            pm = lps.tile([128, B], F32, tag="ps")
            if single_k:
                nc.tensor.matmul(pm[:], W_t[:, 0, ts(m, 128)], rhs, start=True, stop=True)
            else:
                for c in range(HC):
                    nc.tensor.matmul(pm[:], W_t[:, c, ts(m, 128)], rhs[:, c, :],
                                     start=(c == 0), stop=(dual is None and c == HC - 1))
                if dual is not None:
                    W2_t, rhs2, koff = dual
                    for c in range(HC):
                        nc.tensor.matmul(pm[:], W2_t[:, koff + c, ts(m, 128)],
                                         rhs2[:, c, :], start=False, stop=(c == HC - 1))
            psums.append(pm[:])
        out = ac.tile([128, HC, B], F32, tag=out_tag)
        _elu_p1(nc, mne, mne, psums, out, b_a, b_b, B, extra=extra, prepool=mne)
        return out

    # ---- prior (reads h before update; off the critical path) ----
    p1a = hc_layer(W["p1"], h, bias["bp1_a"], bias["bp1_b"], "pact")
    p2a = hc_layer(W["p2"], p1a, bias["bp2_a"], bias["bp2_b"], "pact")
    pr3 = lps.tile([Z_DIM, B], F32, tag="ps")
    _mm(nc, pr3[:], W["p3"], p2a, HC, slice(0, Z_DIM))
    nc.vector.tensor_scalar_add(EP8[Z_DIM:, tl, :], pr3[:], bias["bp3"][:])

    # ---- enc ----
    e1a = hc_layer(W["e1b"], h, None, None, "act", extra=E2[:, :, tl2, :])
    e2a = hc_layer(W["e2"], e1a, bias["be2_a"], bias["be2_b"], "act")
    p3 = lps.tile([Z_DIM, B], F32, tag="ps")
    _mm(nc, p3[:], W["e3"], e2a, HC, slice(0, Z_DIM))
    zx = sm.tile([Z_DIM + 1, B], F32, tag="zx")
    nc.vector.tensor_scalar(zx[:Z_DIM, :], p3[:], bias["th3"][:], None, op0=ALU.is_ge)
    nc.vector.memset(zx[Z_DIM:, :], 1.0)
    nc.vector.tensor_scalar_add(EP8[:Z_DIM, tl, :], p3[:], bias["be3"][:])

    # ---- phi_z ----
    z1a = hc_layer(W["z1"], zx[:], None, None, "act", single_k=True)
    z2a = hc_layer(W["z2"], z1a, bias["bz2_a"], bias["bz2_b"], "act")
    z3a = hc_layer(W["z3"], z2a, bias["bz3_a"], bias["bz3_b"], "z3a")

    # ---- dec ----
    d1a = hc_layer(W["d1"], z3a, bias["bd1_a"], bias["bd1_b"], "act",
                   dual=(W["d1"], h, HC))
    d2a = hc_layer(W["d2"], d1a, bias["bd2_a"], bias["bd2_b"], "act")
    d3a = hc_layer(W["d3"], d2a, bias["bd3_a"], bias["bd3_b"], "act")
    p4 = lps.tile([X_DIM, B], F32, tag="ps")
    _mm(nc, p4[:], W["d4"], d3a, HC, slice(0, X_DIM))
    dx = sm.tile([96, B], F32, tag="dx")
    nc.vector.memset(dx[64:96, :], 1.0)
    nc.vector.tensor_scalar_add(dx[:X_DIM, :], p4[:], bias["bd4"][:])
    nc.sync.dma_start(DEC[ds(si, 1), tl, :, :].rearrange("o p b -> p (o b)"),
                      dx[:X_DIM, :])

    # ---- phi_x_gen ----
    x1a = hc_layer(W["x1"], dx[:X_DIM + 1, :], None, None, "act", single_k=True)
    x2a = hc_layer(W["x2"], x1a, bias["bx2_a"], bias["bx2_b"], "act")
    x3a = hc_layer(W["x3"], x2a, bias["bx3_a"], bias["bx3_b"], "x3a")

    # ---- GRU matmuls ----
    gi = gps.tile([128, 8, B], F32, tag="gi")
    gn = lps.tile([128, HC, B], F32, tag="ps")
    hn = lps.tile([128, HC, B], F32, tag="ps")
    for m in range(8):
        for c in range(HC):
            nc.tensor.matmul(gi[:, m, :], W["ih"][:, c, ts(m, 128)], x3a[:, c, :],
                             start=(c == 0), stop=False)
        for c in range(HC):
            nc.tensor.matmul(gi[:, m, :], W["ih"][:, HC + c, ts(m, 128)], z3a[:, c, :],
                             start=False, stop=False)
        for c in range(HC):
            nc.tensor.matmul(gi[:, m, :], W["hh"][:, c, ts(m, 128)], h[:, c, :],
                             start=False, stop=(c == HC - 1))
    for j in range(HC):
        for c in range(HC):
            nc.tensor.matmul(gn[:, j, :], W["ih"][:, c, ts(8 + j, 128)], x3a[:, c, :],
                             start=(c == 0), stop=False)
        for c in range(HC):
            nc.tensor.matmul(gn[:, j, :], W["ih"][:, HC + c, ts(8 + j, 128)], z3a[:, c, :],
                             start=False, stop=(c == HC - 1))
        for c in range(HC):
            nc.tensor.matmul(hn[:, j, :], W["hh"][:, c, ts(8 + j, 128)], h[:, c, :],
                             start=(c == 0), stop=(c == HC - 1))

    # ---- gates: r and z via sigmoid = 1/(1+exp(-pre)) ----
    def gate_rz(lo, out_tag):
        pre = sm.tile([128, HC, B], F32, tag="rzp")
        brz_v = brz_t[:, lo:lo + HC, None].broadcast_to([128, HC, B])
        nc.vector.tensor_tensor(pre[:], gi[:, lo:lo + HC, :], brz_v, op=ALU.add)
        nc.scalar.activation(pre[:], pre[:], AF.Exp, scale=-1.0)
        nc.vector.tensor_scalar_add(pre[:], pre[:], 1.0)
        out = sm.tile([128, HC, B], F32, tag=out_tag)
        nc.vector.reciprocal(out[:], pre[:])
        return out

    r = gate_rz(0, "r")
    z = gate_rz(HC, "z")

    # n = tanh(gn + bin + r * (hn + bhn)); tanh(x) = 2/(1+exp(-2x)) - 1
    t1 = sm.tile([128, HC, B], F32, tag="gA")
    bhn_v = bhn_t[:, :, None].broadcast_to([128, HC, B])
    nc.vector.tensor_tensor(t1[:], hn[:], bhn_v, op=ALU.add)
    nc.vector.tensor_tensor(t1[:], t1[:], r[:], op=ALU.mult)
    t2 = sm.tile([128, HC, B], F32, tag="gB")
    for m in range(HC):
        nc.vector.scalar_tensor_tensor(t2[:, m, :], gn[:, m, :], 0.0, t1[:, m, :],
                                       op0=ALU.add, op1=ALU.add)
    bin_v = bin_t[:, :, None].broadcast_to([128, HC, B])
    nc.vector.tensor_tensor(t2[:], t2[:], bin_v, op=ALU.add)
    en = sm.tile([128, HC, B], F32, tag="gA")
    nc.scalar.activation(en[:], t2[:], AF.Exp, scale=-2.0)
    nc.vector.tensor_scalar_add(en[:], en[:], 1.0)
    nt = sm.tile([128, HC, B], F32, tag="nt")
    nc.vector.reciprocal(nt[:], en[:])
    nc.vector.tensor_scalar(nt[:], nt[:], 2.0, -1.0, op0=ALU.mult, op1=ALU.add)

    # h = n + z*(h - n)
    dh = sm.tile([128, HC, B], F32, tag="gB")
    nc.vector.tensor_tensor(dh[:], h[:], nt[:], op=ALU.subtract)
    nc.vector.tensor_tensor(dh[:], dh[:], z[:], op=ALU.mult)
    nc.vector.tensor_tensor(h[:], nt[:], dh[:], op=ALU.add)


def _emit_kld(nc, tc, ENCP, PRIP, KLD, NS):
    SC = max(dv for dv in range(1, min(NS, 5) + 1) if NS % dv == 0)
    KCH = NS // SC
    NB = SC * 8 * B
    with (
        tc.tile_pool(name="ka", bufs=1) as ka,
        tc.tile_pool(name="kc", bufs=1) as kc,
    ):
        acc = kc.tile([Z_DIM, 1], F32, tag="acc")
        nc.vector.memset(acc[:], 0.0)

        def sig(u, tg):
            e = ka.tile([Z_DIM, NB], F32, tag="sge")
            nc.scalar.activation(e[:], u[:], AF.Exp, scale=-1.0)
            nc.vector.tensor_scalar_add(e[:], e[:], 1.0)
            o = ka.tile([Z_DIM, NB], F32, tag=tg)
            nc.vector.reciprocal(o[:], e[:])
            return o

        def lnclip(x, tg):
            c = ka.tile([Z_DIM, NB], F32, tag="lncs")
            nc.vector.tensor_scalar_max(c[:], x[:], 0.001)
            o = ka.tile([Z_DIM, NB], F32, tag=tg)
            nc.scalar.activation(o[:], c[:], AF.Ln)
            return o

        for k in range(KCH):
            up = ka.tile([Z_DIM, NB], F32, tag="up")
            nc.sync.dma_start(up[:], ENCP[:, k * SC:(k + 1) * SC, :]
                              .rearrange("p s x -> p (s x)"))
            vp = ka.tile([Z_DIM, NB], F32, tag="vp")
            nc.sync.dma_start(vp[:], PRIP[:, k * SC:(k + 1) * SC, :]
                              .rearrange("p s x -> p (s x)"))
            enc = sig(up, "enc")
            pri = sig(vp, "pri")
            ome = ka.tile([Z_DIM, NB], F32, tag="om")
            nc.vector.tensor_scalar(ome[:], enc[:], -1.0, 1.0, op0=ALU.mult, op1=ALU.add)
            omp = ka.tile([Z_DIM, NB], F32, tag="om2")
            nc.vector.tensor_scalar(omp[:], pri[:], -1.0, 1.0, op0=ALU.mult, op1=ALU.add)
            l1 = lnclip(enc, "l1")
            l2 = lnclip(pri, "l2")
            l3 = lnclip(ome, "l3")
            l4 = lnclip(omp, "l4")
            nc.vector.tensor_tensor(l1[:], l1[:], l2[:], op=ALU.subtract)
            nc.vector.tensor_tensor(l3[:], l3[:], l4[:], op=ALU.subtract)
            nc.vector.tensor_tensor(l1[:], enc[:], l1[:], op=ALU.mult)
            nc.vector.tensor_tensor(l3[:], ome[:], l3[:], op=ALU.mult)
            nc.vector.tensor_tensor(l1[:], l1[:], l3[:], op=ALU.add)
            part = ka.tile([Z_DIM, 1], F32, tag="pt")
            nc.vector.tensor_reduce(part[:], l1[:], axis=mybir.AxisListType.X, op=ALU.add)
            nc.vector.tensor_tensor(acc[:], acc[:], part[:], op=ALU.add)
        nc.sync.dma_start(KLD[:], acc[:])


def kernel(**inputs):
    T = inputs["y"].shape[1]
    prepped = _prep_inputs(
        inputs["y"], inputs["mean_mel"], inputs["std_mel"], inputs["phi_x_params"],
        inputs["phi_z_params"], inputs["enc_params"], inputs["prior_params"],
        inputs["dec_params"], inputs["gru_params"])
    if T not in _CACHE:
        _CACHE[T] = _build(T)
    nc = _CACHE[T]
    res = run_bass_kernel_spmd(nc, [prepped] * 8, list(range(8))).results
    dec = res[0]["DEC"].reshape(T, X_DIM, B).transpose(2, 0, 1)
    kld = np.float32(res[0]["KLD"].sum() / (T * B))
    return np.ascontiguousarray(dec), kld


# revision 12
# speedup vs baseline: 54.0709x; 54.0709x over previous
"""Bass/Tile kernel for nn_BVRNN: GRU-based variational RNN forward on trn2.

The recurrence is strictly sequential in T with batch 64, so the recurrent
loop runs on a single core (SPMD-replicated across all 8; core 0's result is
used). All matmuls are weight-stationary (lhsT = W tile) with activations
kept feature-major [feat_partition, batch_free]. The phi_x MLP and the phi_x
half of enc layer 1 are precomputed for all (b, t) before the loop (the "E"
contribution), stored t-major in DRAM and streamed per 8-step group.

ELU outputs are stored in '+1' form (elu(x)+1 = max(pre+1, exp(min(pre, 0)))),
with the -1 correction folded into downstream biases via column sums.
z = round(sigmoid(u)) is computed as (u >= -b) directly from the preact psum.
Only Exp/Ln ACT functions are used; sigmoid/tanh are emulated with
exp + reciprocal. kld is computed in a vectorized post-pass from staged
enc/prior preactivations.
"""

import numpy as np

import concourse.bass as bass
import concourse.mybir as mybir
import concourse.tile as tile
from concourse import bacc
from concourse.bass import ds, ts
from concourse.bass_utils import run_bass_kernel_spmd
from concourse.masks import make_identity

F32 = mybir.dt.float32
AF = mybir.ActivationFunctionType
ALU = mybir.AluOpType

X_DIM, H_DIM, Z_DIM = 80, 512, 64
B = 64
HC = H_DIM // 128  # 4

_CACHE = {}


def _colsum(W):
    return np.asarray(W, np.float32).sum(axis=0)


def _pc(b, C):
    b = np.asarray(b, np.float32)
    return np.ascontiguousarray(b.reshape(C, 128).T)


def _bc(b, C):
    p = _pc(b, C)
    return np.ascontiguousarray(np.repeat(p[:, :, None], B, axis=2).reshape(128, C * B))


def _prep_inputs(y, mean_mel, std_mel, phi_x_params, phi_z_params, enc_params,
                 prior_params, dec_params, gru_params):
    f32 = lambda x: np.ascontiguousarray(np.asarray(x, np.float32))
    (Wx1, bx1), (Wx2, bx2), (Wx3, bx3) = [(f32(w), f32(b)) for w, b in phi_x_params]
    (Wz1, bz1), (Wz2, bz2), (Wz3, bz3) = [(f32(w), f32(b)) for w, b in phi_z_params]
    (We1, be1), (We2, be2), (We3, be3) = [(f32(w), f32(b)) for w, b in enc_params]
    (Wp1, bp1), (Wp2, bp2), (Wp3, bp3) = [(f32(w), f32(b)) for w, b in prior_params]
    (Wd1, bd1), (Wd2, bd2), (Wd3, bd3), (Wd4, bd4) = [(f32(w), f32(b)) for w, b in dec_params]
    w_ih, w_hh, b_ih, b_hh = [f32(x) for x in gru_params]
    mean = f32(mean_mel)
    std = f32(std_mel)

    Wx1n = Wx1 / std[:, None]
    bx1n = bx1 - (mean / std) @ Wx1

    bx2p = bx2 - _colsum(Wx2)
    bx3p = bx3 - _colsum(Wx3)
    beE = be1 - _colsum(We1[:H_DIM])
    be2p = be2 - _colsum(We2)
    be3p = be3 - _colsum(We3)
    bz2p = bz2 - _colsum(Wz2)
    bz3p = bz3 - _colsum(Wz3)
    bd1p = bd1 - _colsum(Wd1[:H_DIM])
    bd2p = bd2 - _colsum(Wd2)
    bd3p = bd3 - _colsum(Wd3)
    bd4p = bd4 - _colsum(Wd4)
    b_ihp = b_ih - _colsum(w_ih[:H_DIM]) - _colsum(w_ih[H_DIM:])
    bp2p = bp2 - _colsum(Wp2)
    bp3p = bp3 - _colsum(Wp3)

    Wz1e = np.concatenate([Wz1, bz1[None, :]], axis=0)       # [65, 512]
    Wx1e = np.concatenate([Wx1n, bx1n[None, :]], axis=0)     # [81, 512]

    brz = (b_ihp + b_hh)[: 2 * H_DIM]
    bin_ = b_ihp[2 * H_DIM:]
    bhn = b_hh[2 * H_DIM:]

    return {
        "y": f32(y),
        "Wx1n": Wx1n, "bx1n_a": _pc(bx1n, HC), "bx1n_b": _pc(bx1n + 1.0, HC),
        "Wx2": Wx2, "bx2_a": _pc(bx2p, HC), "bx2_b": _pc(bx2p + 1.0, HC),
        "Wx3": Wx3, "bx3_a": _pc(bx3p, HC), "bx3_b": _pc(bx3p + 1.0, HC),
        "We1t": np.ascontiguousarray(We1[:H_DIM]),
        "We1b": np.ascontiguousarray(We1[H_DIM:]),
        "beE": _pc(beE, HC),
        "We2": We2, "be2_a": _pc(be2p, HC), "be2_b": _pc(be2p + 1.0, HC),
        "We3": We3, "be3": np.ascontiguousarray(be3p.reshape(Z_DIM, 1)),
        "th3": np.ascontiguousarray((-be3p).reshape(Z_DIM, 1)),
        "Wz1e": Wz1e,
        "Wz2": Wz2, "bz2_a": _pc(bz2p, HC), "bz2_b": _pc(bz2p + 1.0, HC),
        "Wz3": Wz3, "bz3_a": _pc(bz3p, HC), "bz3_b": _pc(bz3p + 1.0, HC),
        "Wd1": Wd1, "bd1_a": _pc(bd1p, HC), "bd1_b": _pc(bd1p + 1.0, HC),
        "Wd2": Wd2, "bd2_a": _pc(bd2p, HC), "bd2_b": _pc(bd2p + 1.0, HC),
        "Wd3": Wd3, "bd3_a": _pc(bd3p, HC), "bd3_b": _pc(bd3p + 1.0, HC),
        "Wd4": Wd4, "bd4": np.ascontiguousarray(bd4p.reshape(X_DIM, 1)),
        "Wx1e": Wx1e,
        "Wih": w_ih, "Whh": w_hh,
        "brz8": _pc(brz, 8), "bin4": _pc(bin_, HC), "bhn4": _pc(bhn, HC),
        "Wp1": Wp1, "bp1_a": _pc(bp1, HC), "bp1_b": _pc(bp1 + 1.0, HC),
        "Wp2": Wp2, "bp2_a": _pc(bp2p, HC), "bp2_b": _pc(bp2p + 1.0, HC),
        "Wp3": Wp3, "bp3": np.ascontiguousarray(bp3p.reshape(Z_DIM, 1)),
    }


def _load_w(nc, pool, dram, K, M):
    """Load W [K, M] into SBUF tile [128 or K, K//128 or 1, M]."""
    tag = "w_" + dram.name
    if K > 128:
        assert K % 128 == 0
        t = pool.tile([128, K // 128, M], F32, tag=tag)
        nc.sync.dma_start(t[:], dram[:].rearrange("(c p) m -> p c m", p=128))
    else:
        t = pool.tile([K, 1, M], F32, tag=tag)
        nc.sync.dma_start(t[:, 0, :], dram[:])
    return t


def _ld_bias(nc, pool, dram, shape):
    t = pool.tile(list(shape), F32, tag="b_" + dram.name)
    nc.sync.dma_start(t[:], dram[:])
    return t


def _elu_p1(nc, mnpool, epool, psums, out, b_a, b_b, nb, extra=None, prepool=None):
    """out[:, m, :] = elu(psum_m + bias_m [+ extra_m]) + 1.

    psums: list of per-m psum APs [128, nb]. b_a/b_b: [128, C] tiles or None
    (bias already included; uses 0/+1 immediates). extra: [128, C, nb] sbuf AP
    added to the preact (enc1's E contribution).
    """
    C = len(psums)
    mn = mnpool.tile([128, C, nb], F32, tag="mn")
    if extra is not None:
        pre = prepool.tile([128, C, nb], F32, tag="pre")
        for m in range(C):
            nc.vector.scalar_tensor_tensor(pre[:, m, :], psums[m], 0.0,
                                           extra[:, m, :], op0=ALU.add, op1=ALU.add)
        for m in range(C):
            nc.vector.tensor_scalar(mn[:, m, :], pre[:, m, :], 0.0, None, op0=ALU.min)
    elif b_a is not None:
        for m in range(C):
            nc.vector.tensor_scalar(mn[:, m, :], psums[m], b_a[:, m:m + 1], 0.0,
                                    op0=ALU.add, op1=ALU.min)
    else:
        for m in range(C):
            nc.vector.tensor_scalar(mn[:, m, :], psums[m], 0.0, None, op0=ALU.min)
    e = epool.tile([128, C, nb], F32, tag="ee")
    nc.scalar.activation(e[:], mn[:], AF.Exp)
    for m in range(C):
        if extra is not None:
            nc.vector.scalar_tensor_tensor(out[:, m, :], pre[:, m, :], 1.0,
                                           e[:, m, :], op0=ALU.add, op1=ALU.max)
        elif b_b is not None:
            nc.vector.scalar_tensor_tensor(out[:, m, :], psums[m], b_b[:, m:m + 1],
                                           e[:, m, :], op0=ALU.add, op1=ALU.max)
        else:
            nc.vector.scalar_tensor_tensor(out[:, m, :], psums[m], 1.0,
                                           e[:, m, :], op0=ALU.add, op1=ALU.max)


def _build(T):
    assert T % 8 == 0
    NS = T // 8

    nc = bacc.Bacc("TRN2", target_bir_lowering=False, debug=False, num_devices=8)

    d = {}
    def din(name, shape):
        d[name] = nc.dram_tensor(name, list(shape), F32, kind="ExternalInput")
    din("y", (B, T, X_DIM))
    din("Wx1n", (X_DIM, H_DIM)); din("bx1n_a", (128, HC)); din("bx1n_b", (128, HC))
    din("Wx2", (H_DIM, H_DIM)); din("bx2_a", (128, HC)); din("bx2_b", (128, HC))
    din("Wx3", (H_DIM, H_DIM)); din("bx3_a", (128, HC)); din("bx3_b", (128, HC))
    din("We1t", (H_DIM, H_DIM)); din("We1b", (H_DIM, H_DIM)); din("beE", (128, HC))
    din("We2", (H_DIM, H_DIM)); din("be2_a", (128, HC)); din("be2_b", (128, HC))
    din("We3", (H_DIM, Z_DIM)); din("be3", (Z_DIM, 1)); din("th3", (Z_DIM, 1))
    din("Wz1e", (Z_DIM + 1, H_DIM))
    din("Wz2", (H_DIM, H_DIM)); din("bz2_a", (128, HC)); din("bz2_b", (128, HC))
    din("Wz3", (H_DIM, H_DIM)); din("bz3_a", (128, HC)); din("bz3_b", (128, HC))
    din("Wd1", (2 * H_DIM, H_DIM)); din("bd1_a", (128, HC)); din("bd1_b", (128, HC))
    din("Wd2", (H_DIM, H_DIM)); din("bd2_a", (128, HC)); din("bd2_b", (128, HC))
    din("Wd3", (H_DIM, H_DIM)); din("bd3_a", (128, HC)); din("bd3_b", (128, HC))
    din("Wd4", (H_DIM, X_DIM)); din("bd4", (X_DIM, 1))
    din("Wx1e", (X_DIM + 1, H_DIM))
    din("Wih", (2 * H_DIM, 3 * H_DIM)); din("Whh", (H_DIM, 3 * H_DIM))
    din("brz8", (128, 8)); din("bin4", (128, HC)); din("bhn4", (128, HC))
    din("Wp1", (H_DIM, H_DIM)); din("bp1_a", (128, HC)); din("bp1_b", (128, HC))
    din("Wp2", (H_DIM, H_DIM)); din("bp2_a", (128, HC)); din("bp2_b", (128, HC))
    din("Wp3", (H_DIM, Z_DIM)); din("bp3", (Z_DIM, 1))

    DEC = nc.dram_tensor("DEC", [NS, 8, X_DIM, B], F32, kind="ExternalOutput")
    KLD = nc.dram_tensor("KLD", [Z_DIM, 1], F32, kind="ExternalOutput")

    Eflat = nc.dram_tensor("Eflat", [NS, 128, HC * 8 * B], F32)
    ENCP = nc.dram_tensor("ENCP", [Z_DIM, NS, 8 * B], F32)
    PRIP = nc.dram_tensor("PRIP", [Z_DIM, NS, 8 * B], F32)

    with tile.TileContext(nc) as tc:
        _emit_precompute(nc, tc, d, Eflat, T)
        _emit_loop(nc, tc, d, DEC, Eflat, ENCP, PRIP, NS)
        _emit_kld(nc, tc, ENCP, PRIP, KLD, NS)

    nc.compile()
    return nc


def _emit_precompute(nc, tc, d, Eflat, T):
    TCH = 120 if T % 120 == 0 else 8
    NCH = T // TCH
    ROWS = B * TCH
    assert ROWS % 512 == 0
    NSL = ROWS // 512
    TPS = 512 // B  # t's per slice = 8
    with (
        tc.tile_pool(name="pw", bufs=1) as pw,
        tc.tile_pool(name="pa", bufs=2) as pa,
        tc.tile_pool(name="pyt", bufs=2) as pyt,
        tc.tile_pool(name="pps", bufs=6, space="PSUM") as pps,
        tc.tile_pool(name="ppt", bufs=2, space="PSUM") as ppt,
    ):
        ident = pw.tile([128, 128], F32, tag="ident")
        make_identity(nc, ident[:])
        Wx1n_t = _load_w(nc, pw, d["Wx1n"], X_DIM, H_DIM)
        Wx2_t = _load_w(nc, pw, d["Wx2"], H_DIM, H_DIM)
        Wx3_t = _load_w(nc, pw, d["Wx3"], H_DIM, H_DIM)
        We1t_t = _load_w(nc, pw, d["We1t"], H_DIM, H_DIM)
        bb = {}
        for nm in ["bx1n_a", "bx1n_b", "bx2_a", "bx2_b", "bx3_a", "bx3_b", "beE"]:
            bb[nm] = _ld_bias(nc, pw, d[nm], (128, HC))

        for ch in range(NCH):
            yT = pyt.tile([X_DIM, ROWS], F32, tag="yT")
            for b in range(B):
                yblk = pa.tile([TCH, X_DIM], F32, tag="yblk")
                nc.sync.dma_start(yblk[:], d["y"][b, ch * TCH:(ch + 1) * TCH, :])
                tp = ppt.tile([X_DIM, TCH], F32, tag="tp")
                nc.tensor.transpose(tp[:], yblk[:], ident[:TCH, :TCH])
                nc.vector.tensor_copy(yT[:, b * TCH:(b + 1) * TCH], tp[:])

            yTv = yT[:].rearrange("p (b t) -> p t b", b=B)
            for s in range(NSL):
                rhs1 = yTv[:, s * TPS:(s + 1) * TPS, :]

                def layer(W_t, rhs, KC, b_a, b_b, out_tag, single_k=False):
                    psums = []
                    for m in range(HC):
                        pm = pps.tile([128, 512], F32, tag="ps")
                        if single_k:
                            nc.tensor.matmul(pm[:], W_t[:, 0, ts(m, 128)], rhs,
                                             start=True, stop=True)
                        else:
                            for c in range(HC):
                                nc.tensor.matmul(pm[:], W_t[:, c, ts(m, 128)],
                                                 rhs[:, c, :], start=(c == 0),
                                                 stop=(c == HC - 1))
                        psums.append(pm[:])
                    out = pa.tile([128, HC, 512], F32, tag=out_tag)
                    _elu_p1(nc, pa, pa, psums, out, b_a, b_b, 512)
                    return out

                a1 = layer(Wx1n_t, rhs1, 1, bb["bx1n_a"], bb["bx1n_b"], "a1", single_k=True)
                a2 = layer(Wx2_t, a1, HC, bb["bx2_a"], bb["bx2_b"], "a2")
                a3 = layer(Wx3_t, a2, HC, bb["bx3_a"], bb["bx3_b"], "a1")
                eo = pa.tile([128, HC, 512], F32, tag="a2")
                for m in range(HC):
                    pm = pps.tile([128, 512], F32, tag="ps")
                    for c in range(HC):
                        nc.tensor.matmul(pm[:], We1t_t[:, c, ts(m, 128)], a3[:, c, :],
                                         start=(c == 0), stop=(c == HC - 1))
                    nc.vector.tensor_scalar_add(eo[:, m, :], pm[:], bb["beE"][:, m:m + 1])
                gs = (ch * TCH) // TPS + s
                nc.sync.dma_start(
                    Eflat[gs, :, :].rearrange("p (c x) -> p c x", c=HC), eo[:])


def _emit_loop(nc, tc, d, DEC, Eflat, ENCP, PRIP, NS):
    with (
        tc.tile_pool(name="lw", bufs=1) as lw,
        tc.tile_pool(name="st", bufs=1) as st,
        tc.tile_pool(name="io", bufs=1) as io,
        tc.tile_pool(name="e8", bufs=2) as e8p,
        tc.tile_pool(name="ac", bufs=2) as ac,
        tc.tile_pool(name="sm", bufs=1) as sm,
        tc.tile_pool(name="mne", bufs=1) as mne,
        tc.tile_pool(name="lps", bufs=6, space="PSUM") as lps,
        tc.tile_pool(name="gps", bufs=2, space="PSUM") as gps,
    ):
        W = {}
        W["e1b"] = _load_w(nc, lw, d["We1b"], H_DIM, H_DIM)
        W["e2"] = _load_w(nc, lw, d["We2"], H_DIM, H_DIM)
        W["e3"] = _load_w(nc, lw, d["We3"], H_DIM, Z_DIM)
        W["z1"] = _load_w(nc, lw, d["Wz1e"], Z_DIM + 1, H_DIM)
        W["z2"] = _load_w(nc, lw, d["Wz2"], H_DIM, H_DIM)
        W["z3"] = _load_w(nc, lw, d["Wz3"], H_DIM, H_DIM)
        W["d1"] = _load_w(nc, lw, d["Wd1"], 2 * H_DIM, H_DIM)
        W["d2"] = _load_w(nc, lw, d["Wd2"], H_DIM, H_DIM)
        W["d3"] = _load_w(nc, lw, d["Wd3"], H_DIM, H_DIM)
        W["d4"] = _load_w(nc, lw, d["Wd4"], H_DIM, X_DIM)
        W["x1"] = _load_w(nc, lw, d["Wx1e"], X_DIM + 1, H_DIM)
        W["x2"] = _load_w(nc, lw, d["Wx2"], H_DIM, H_DIM)
        W["x3"] = _load_w(nc, lw, d["Wx3"], H_DIM, H_DIM)
        W["ih"] = _load_w(nc, lw, d["Wih"], 2 * H_DIM, 3 * H_DIM)
        W["hh"] = _load_w(nc, lw, d["Whh"], H_DIM, 3 * H_DIM)
        W["p1"] = _load_w(nc, lw, d["Wp1"], H_DIM, H_DIM)
        W["p2"] = _load_w(nc, lw, d["Wp2"], H_DIM, H_DIM)
        W["p3"] = _load_w(nc, lw, d["Wp3"], H_DIM, Z_DIM)

        bias = {}
        for nm in ["bx2", "bx3", "be2", "bz2", "bz3", "bd1", "bd2", "bd3",
                   "bp1", "bp2"]:
            bias[nm + "_a"] = _ld_bias(nc, lw, d[nm + "_a"], (128, HC))
            bias[nm + "_b"] = _ld_bias(nc, lw, d[nm + "_b"], (128, HC))
        bias["be3"] = _ld_bias(nc, lw, d["be3"], (Z_DIM, 1))
        bias["th3"] = _ld_bias(nc, lw, d["th3"], (Z_DIM, 1))
        bias["bd4"] = _ld_bias(nc, lw, d["bd4"], (X_DIM, 1))
        bias["bp3"] = _ld_bias(nc, lw, d["bp3"], (Z_DIM, 1))
        brz_t = _ld_bias(nc, lw, d["brz8"], (128, 8))
        bin_t = _ld_bias(nc, lw, d["bin4"], (128, HC))
        bhn_t = _ld_bias(nc, lw, d["bhn4"], (128, HC))

        h = st.tile([128, HC, B], F32, tag="h")
        nc.vector.memset(h[:], 0.0)

        with tc.For_i(0, NS, 1, hint_engines=tuple(mybir.ALL_ENGINES)) as si:
            Ev = Eflat[ds(si, 1), :, :].rearrange("o p (c t b) -> p c (o t) b",
                                                  c=HC, b=B)
            EP8 = io.tile([128, 8, B], F32, tag="EP8")  # enc preacts [:64], prior [64:]
            for half in range(4):
                E2 = e8p.tile([128, HC, 2, B], F32, tag="E2")
                nc.sync.dma_start(E2[:], Ev[:, :, half * 2:(half + 1) * 2, :])
                for tl2 in range(2):
                    tl = half * 2 + tl2
                    _emit_step(nc, ac, sm, mne, lps, gps, h, E2, EP8, DEC, si, tl,
                               tl2, W, bias, brz_t, bin_t, bhn_t)

            nc.sync.dma_start(
                ENCP[:, ds(si, 1), :].rearrange("p o x -> p (o x)"),
                EP8[:Z_DIM, :, :].rearrange("p t b -> p (t b)"))
            nc.sync.dma_start(
                PRIP[:, ds(si, 1), :].rearrange("p o x -> p (o x)"),
                EP8[Z_DIM:, :, :].rearrange("p t b -> p (t b)"))


def _mm(nc, psum_ap, W_t, rhs, KC, mslice, start=True, stop=True):
    for c in range(KC):
        nc.tensor.matmul(psum_ap, W_t[:, c, mslice], rhs[:, c, :],
                         start=(start and c == 0), stop=(stop and c == KC - 1))


def _emit_step(nc, ac, sm, mne, lps, gps, h, E2, EP8, DEC, si, tl, tl2,
               W, bias, brz_t, bin_t, bhn_t):
    def hc_layer(W_t, rhs, b_a, b_b, out_tag, single_k=False, extra=None,
                 dual=None):
        psums = []
        for m in range(HC):
            pm = lps.tile([128, B], F32, tag="ps")
            if single_k:
                nc.tensor.matmul(pm[:], W_t[:, 0, ts(m, 128)], rhs, start=True, stop=True)
            else:
                for c in range(HC):
                    nc.tensor.matmul(pm[:], W_t[:, c, ts(m, 128)], rhs[:, c, :],
                                     start=(c == 0), stop=(dual is None and c == HC - 1))
                if dual is not None:
                    W2_t, rhs2, koff = dual
                    for c in range(HC):
                        nc.tensor.matmul(pm[:], W2_t[:, koff + c, ts(m, 128)],
                                         rhs2[:, c, :], start=False, stop=(c == HC - 1))
            psums.append(pm[:])
        out = ac.tile([128, HC, B], F32, tag=out_tag)
        _elu_p1(nc, mne, mne, psums, out, b_a, b_b, B, extra=extra, prepool=mne)
        return out

    # ---- prior (reads h before update; off the critical path) ----
    p1a = hc_layer(W["p1"], h, bias["bp1_a"], bias["bp1_b"], "pact")
    p2a = hc_layer(W["p2"], p1a, bias["bp2_a"], bias["bp2_b"], "pact")
    pr3 = lps.tile([Z_DIM, B], F32, tag="ps")
    _mm(nc, pr3[:], W["p3"], p2a, HC, slice(0, Z_DIM))
    nc.vector.tensor_scalar_add(EP8[Z_DIM:, tl, :], pr3[:], bias["bp3"][:])

    # ---- enc ----
    e1a = hc_layer(W["e1b"], h, None, None, "act", extra=E2[:, :, tl2, :])
    e2a = hc_layer(W["e2"], e1a, bias["be2_a"], bias["be2_b"], "act")
    p3 = lps.tile([Z_DIM, B], F32, tag="ps")
    _mm(nc, p3[:], W["e3"], e2a, HC, slice(0, Z_DIM))
    zx = sm.tile([Z_DIM + 1, B], F32, tag="zx")
    nc.vector.tensor_scalar(zx[:Z_DIM, :], p3[:], bias["th3"][:], None, op0=ALU.is_ge)
    nc.vector.memset(zx[Z_DIM:, :], 1.0)
    nc.vector.tensor_scalar_add(EP8[:Z_DIM, tl, :], p3[:], bias["be3"][:])

    # ---- phi_z ----
    z1a = hc_layer(W["z1"], zx[:], None, None, "act", single_k=True)
    z2a = hc_layer(W["z2"], z1a, bias["bz2_a"], bias["bz2_b"], "act")
    z3a = hc_layer(W["z3"], z2a, bias["bz3_a"], bias["bz3_b"], "z3a")

    # ---- dec ----
    d1a = hc_layer(W["d1"], z3a, bias["bd1_a"], bias["bd1_b"], "act",
                   dual=(W["d1"], h, HC))
    d2a = hc_layer(W["d2"], d1a, bias["bd2_a"], bias["bd2_b"], "act")
    d3a = hc_layer(W["d3"], d2a, bias["bd3_a"], bias["bd3_b"], "act")
    p4 = lps.tile([X_DIM, B], F32, tag="ps")
    _mm(nc, p4[:], W["d4"], d3a, HC, slice(0, X_DIM))
    dx = sm.tile([96, B], F32, tag="dx")
    nc.vector.memset(dx[64:96, :], 1.0)
    nc.vector.tensor_scalar_add(dx[:X_DIM, :], p4[:], bias["bd4"][:])
    nc.sync.dma_start(DEC[ds(si, 1), tl, :, :].rearrange("o p b -> p (o b)"),
                      dx[:X_DIM, :])

    # ---- phi_x_gen ----
    x1a = hc_layer(W["x1"], dx[:X_DIM + 1, :], None, None, "act", single_k=True)
    x2a = hc_layer(W["x2"], x1a, bias["bx2_a"], bias["bx2_b"], "act")
    x3a = hc_layer(W["x3"], x2a, bias["bx3_a"], bias["bx3_b"], "x3a")

    # ---- GRU matmuls ----
    gi = gps.tile([128, 8, B], F32, tag="gi")
    gn = lps.tile([128, HC, B], F32, tag="ps")
    hn = lps.tile([128, HC, B], F32, tag="ps")
    for m in range(8):
        for c in range(HC):
            nc.tensor.matmul(gi[:, m, :], W["ih"][:, c, ts(m, 128)], x3a[:, c, :],
                             start=(c == 0), stop=False)
        for c in range(HC):
            nc.tensor.matmul(gi[:, m, :], W["ih"][:, HC + c, ts(m, 128)], z3a[:, c, :],
                             start=False, stop=False)
        for c in range(HC):
            nc.tensor.matmul(gi[:, m, :], W["hh"][:, c, ts(m, 128)], h[:, c, :],
                             start=False, stop=(c == HC - 1))
    for j in range(HC):
        for c in range(HC):
            nc.tensor.matmul(gn[:, j, :], W["ih"][:, c, ts(8 + j, 128)], x3a[:, c, :],
                             start=(c == 0), stop=False)
        for c in range(HC):
            nc.tensor.matmul(gn[:, j, :], W["ih"][:, HC + c, ts(8 + j, 128)], z3a[:, c, :],
                             start=False, stop=(c == HC - 1))
        for c in range(HC):
            nc.tensor.matmul(hn[:, j, :], W["hh"][:, c, ts(8 + j, 128)], h[:, c, :],
                             start=(c == 0), stop=(c == HC - 1))

    # ---- gates: r and z via sigmoid = 1/(1+exp(-pre)) ----
    def gate_rz(lo, out_tag):
        pre = sm.tile([128, HC, B], F32, tag="rzp")
        brz_v = brz_t[:, lo:lo + HC, None].broadcast_to([128, HC, B])
        nc.vector.tensor_tensor(pre[:], gi[:, lo:lo + HC, :], brz_v, op=ALU.add)
        nc.scalar.activation(pre[:], pre[:], AF.Exp, scale=-1.0)
        nc.vector.tensor_scalar_add(pre[:], pre[:], 1.0)
        out = sm.tile([128, HC, B], F32, tag=out_tag)
        nc.vector.reciprocal(out[:], pre[:])
        return out

    r = gate_rz(0, "r")
    z = gate_rz(HC, "z")

    # n = tanh(gn + bin + r * (hn + bhn)); tanh(x) = 2/(1+exp(-2x)) - 1
    t1 = sm.tile([128, HC, B], F32, tag="gA")
    bhn_v = bhn_t[:, :, None].broadcast_to([128, HC, B])
    nc.vector.tensor_tensor(t1[:], hn[:], bhn_v, op=ALU.add)
    nc.vector.tensor_tensor(t1[:], t1[:], r[:], op=ALU.mult)
    t2 = sm.tile([128, HC, B], F32, tag="gB")
    for m in range(HC):
        nc.vector.scalar_tensor_tensor(t2[:, m, :], gn[:, m, :], 0.0, t1[:, m, :],
                                       op0=ALU.add, op1=ALU.add)
    bin_v = bin_t[:, :, None].broadcast_to([128, HC, B])
    nc.vector.tensor_tensor(t2[:], t2[:], bin_v, op=ALU.add)
    en = sm.tile([128, HC, B], F32, tag="gA")
    nc.scalar.activation(en[:], t2[:], AF.Exp, scale=-2.0)
    nc.vector.tensor_scalar_add(en[:], en[:], 1.0)
    nt = sm.tile([128, HC, B], F32, tag="nt")
    nc.vector.reciprocal(nt[:], en[:])
    nc.vector.tensor_scalar(nt[:], nt[:], 2.0, -1.0, op0=ALU.mult, op1=ALU.add)

    # h = n + z*(h - n)
    dh = sm.tile([128, HC, B], F32, tag="gB")
    nc.vector.tensor_tensor(dh[:], h[:], nt[:], op=ALU.subtract)
    nc.vector.tensor_tensor(dh[:], dh[:], z[:], op=ALU.mult)
    nc.vector.tensor_tensor(h[:], nt[:], dh[:], op=ALU.add)


def _emit_kld(nc, tc, ENCP, PRIP, KLD, NS):
    SC = max(dv for dv in range(1, min(NS, 5) + 1) if NS % dv == 0)
    KCH = NS // SC
    NB = SC * 8 * B
    with (
        tc.tile_pool(name="ka", bufs=1) as ka,
        tc.tile_pool(name="kc", bufs=1) as kc,
    ):
        acc = kc.tile([Z_DIM, 1], F32, tag="acc")
        nc.vector.memset(acc[:], 0.0)

        def sig(u, tg):
            e = ka.tile([Z_DIM, NB], F32, tag="sge")
            nc.scalar.activation(e[:], u[:], AF.Exp, scale=-1.0)
            nc.vector.tensor_scalar_add(e[:], e[:], 1.0)
            o = ka.tile([Z_DIM, NB], F32, tag=tg)
            nc.vector.reciprocal(o[:], e[:])
            return o

        def lnclip(x, tg):
            c = ka.tile([Z_DIM, NB], F32, tag="lncs")
            nc.vector.tensor_scalar_max(c[:], x[:], 0.001)
            o = ka.tile([Z_DIM, NB], F32, tag=tg)
            nc.scalar.activation(o[:], c[:], AF.Ln)
            return o

        for k in range(KCH):
            up = ka.tile([Z_DIM, NB], F32, tag="up")
            nc.sync.dma_start(up[:], ENCP[:, k * SC:(k + 1) * SC, :]
                              .rearrange("p s x -> p (s x)"))
            vp = ka.tile([Z_DIM, NB], F32, tag="vp")
            nc.sync.dma_start(vp[:], PRIP[:, k * SC:(k + 1) * SC, :]
                              .rearrange("p s x -> p (s x)"))
            enc = sig(up, "enc")
            pri = sig(vp, "pri")
            ome = ka.tile([Z_DIM, NB], F32, tag="om")
            nc.vector.tensor_scalar(ome[:], enc[:], -1.0, 1.0, op0=ALU.mult, op1=ALU.add)
            omp = ka.tile([Z_DIM, NB], F32, tag="om2")
            nc.vector.tensor_scalar(omp[:], pri[:], -1.0, 1.0, op0=ALU.mult, op1=ALU.add)
            l1 = lnclip(enc, "l1")
            l2 = lnclip(pri, "l2")
            l3 = lnclip(ome, "l3")
            l4 = lnclip(omp, "l4")
            nc.vector.tensor_tensor(l1[:], l1[:], l2[:], op=ALU.subtract)
            nc.vector.tensor_tensor(l3[:], l3[:], l4[:], op=ALU.subtract)
            nc.vector.tensor_tensor(l1[:], enc[:], l1[:], op=ALU.mult)
            nc.vector.tensor_tensor(l3[:], ome[:], l3[:], op=ALU.mult)
            nc.vector.tensor_tensor(l1[:], l1[:], l3[:], op=ALU.add)
            part = ka.tile([Z_DIM, 1], F32, tag="pt")
            nc.vector.tensor_reduce(part[:], l1[:], axis=mybir.AxisListType.X, op=ALU.add)
            nc.vector.tensor_tensor(acc[:], acc[:], part[:], op=ALU.add)
        nc.sync.dma_start(KLD[:], acc[:])


def run_resident(nc, prepped, n_iters=2):
    """Execute with device-resident inputs; returns (results_core0, best_exec_s).

    Mirrors bass2jax.run_bass_via_pjrt's multi-core path but keeps the jitted
    callable and input arrays on device so repeat calls measure device
    execution (+ small dispatch) rather than host->device transfer.
    """
    import time
    import jax
    import jax.numpy as jnp
    from jax.sharding import Mesh, PartitionSpec
    from jax.experimental.shard_map import shard_map
    from concourse import bass2jax as b2j
    from concourse import mybir as mb

    b2j.install_neuronx_cc_hook()
    n_cores = 8
    partition_name = nc.partition_id_tensor.name if nc.partition_id_tensor else None
    in_names, out_names, out_avals = [], [], []
    for alloc in nc.m.functions[0].allocations:
        if not isinstance(alloc, mb.MemoryLocationSet):
            continue
        name = alloc.memorylocations[0].name
        if alloc.kind == "ExternalInput":
            if name != partition_name:
                in_names.append(name)
        elif alloc.kind == "ExternalOutput":
            out_names.append(name)
            out_avals.append(jax.core.ShapedArray(tuple(alloc.tensor_shape),
                                                  mb.dt.np(alloc.dtype)))
    n_params = len(in_names)
    all_in_names = list(in_names) + list(out_names)
    if partition_name is not None:
        all_in_names.append(partition_name)
    donate = tuple(range(n_params, n_params + len(out_names)))

    def _body(*args):
        operands = list(args)
        if partition_name is not None:
            operands.append(b2j.partition_id_tensor())
        return tuple(b2j._bass_exec_p.bind(
            *operands, out_avals=tuple(out_avals), in_names=tuple(all_in_names),
            out_names=tuple(out_names), lowering_input_output_aliases=(),
            sim_require_finite=True, sim_require_nnan=True, nc=nc))

    devices = jax.devices()[:n_cores]
    mesh = Mesh(np.asarray(devices), ("core",))
    nio = n_params + len(out_names)
    sharded = jax.jit(
        shard_map(_body, mesh=mesh, in_specs=(PartitionSpec("core"),) * nio,
                  out_specs=(PartitionSpec("core"),) * len(out_names),
                  check_rep=False),
        donate_argnums=donate, keep_unused=True)

    from jax.sharding import NamedSharding
    shard = NamedSharding(mesh, PartitionSpec("core"))
    concat_in = [jax.device_put(
        np.concatenate([np.asarray(prepped[nm])] * n_cores, axis=0), shard)
        for nm in in_names]

    def zeros():
        return [jax.device_put(
            np.zeros((n_cores * a.shape[0], *a.shape[1:]), a.dtype), shard)
            for a in out_avals]

    # warm-up (compiles)
    out = sharded(*concat_in, *zeros())
    jax.block_until_ready(out)
    best = None
    for _ in range(n_iters):
        z = zeros()
        jax.block_until_ready(z)
        t0 = time.time()
        out = sharded(*concat_in, *z)
        jax.block_until_ready(out)
        dt = time.time() - t0
        best = dt if best is None or dt < best else best
    res = {name: np.asarray(out[i]).reshape(n_cores, *out_avals[i].shape)[0]
           for i, name in enumerate(out_names)}
    return res, best


def kernel(**inputs):
    T = inputs["y"].shape[1]
    prepped = _prep_inputs(
        inputs["y"], inputs["mean_mel"], inputs["std_mel"], inputs["phi_x_params"],
        inputs["phi_z_params"], inputs["enc_params"], inputs["prior_params"],
        inputs["dec_params"], inputs["gru_params"])
    if T not in _CACHE:
        _CACHE[T] = _build(T)
    nc = _CACHE[T]
    res = run_bass_kernel_spmd(nc, [prepped] * 8, list(range(8))).results
    dec = res[0]["DEC"].reshape(T, X_DIM, B).transpose(2, 0, 1)
    kld = np.float32(res[0]["KLD"].sum() / (T * B))
    return np.ascontiguousarray(dec), kld


# revision 13
# speedup vs baseline: 77.5230x; 1.4337x over previous
"""Bass/Tile kernel for nn_BVRNN: GRU-based variational RNN forward on trn2.

The recurrence is strictly sequential in T with batch 64, so the recurrent
loop runs on a single core (SPMD-replicated across all 8; core 0's result is
used). All matmuls are weight-stationary (lhsT = W tile) with activations
kept feature-major [feat_partition, batch_free]. The phi_x MLP and the phi_x
half of enc layer 1 are precomputed for all (b, t) before the loop (the "E"
contribution), stored t-major in DRAM and streamed per 8-step group.

ELU outputs are stored in '+1' form (elu(x)+1 = max(pre+1, exp(min(pre, 0)))),
with the -1 correction folded into downstream biases via column sums.
z = round(sigmoid(u)) is computed as (u >= -b) directly from the preact psum.
Only Exp/Ln ACT functions are used; sigmoid/tanh are emulated with
exp + reciprocal. kld is computed in a vectorized post-pass from staged
enc/prior preactivations.
"""

import numpy as np

import concourse.bass as bass
import concourse.mybir as mybir
import concourse.tile as tile
from concourse import bacc
from concourse.bass import ds, ts
from concourse.bass_utils import run_bass_kernel_spmd
from concourse.masks import make_identity

F32 = mybir.dt.float32
AF = mybir.ActivationFunctionType
ALU = mybir.AluOpType

X_DIM, H_DIM, Z_DIM = 80, 512, 64
B = 64
HC = H_DIM // 128  # 4

_CACHE = {}


def _colsum(W):
    return np.asarray(W, np.float32).sum(axis=0)


def _pc(b, C):
    b = np.asarray(b, np.float32)
    return np.ascontiguousarray(b.reshape(C, 128).T)


def _bc(b, C):
    p = _pc(b, C)
    return np.ascontiguousarray(np.repeat(p[:, :, None], B, axis=2).reshape(128, C * B))


def _prep_inputs(y, mean_mel, std_mel, phi_x_params, phi_z_params, enc_params,
                 prior_params, dec_params, gru_params):
    f32 = lambda x: np.ascontiguousarray(np.asarray(x, np.float32))
    (Wx1, bx1), (Wx2, bx2), (Wx3, bx3) = [(f32(w), f32(b)) for w, b in phi_x_params]
    (Wz1, bz1), (Wz2, bz2), (Wz3, bz3) = [(f32(w), f32(b)) for w, b in phi_z_params]
    (We1, be1), (We2, be2), (We3, be3) = [(f32(w), f32(b)) for w, b in enc_params]
    (Wp1, bp1), (Wp2, bp2), (Wp3, bp3) = [(f32(w), f32(b)) for w, b in prior_params]
    (Wd1, bd1), (Wd2, bd2), (Wd3, bd3), (Wd4, bd4) = [(f32(w), f32(b)) for w, b in dec_params]
    w_ih, w_hh, b_ih, b_hh = [f32(x) for x in gru_params]
    mean = f32(mean_mel)
    std = f32(std_mel)

    Wx1n = Wx1 / std[:, None]
    bx1n = bx1 - (mean / std) @ Wx1

    bx2p = bx2 - _colsum(Wx2)
    bx3p = bx3 - _colsum(Wx3)
    beE = be1 - _colsum(We1[:H_DIM])
    be2p = be2 - _colsum(We2)
    be3p = be3 - _colsum(We3)
    bz2p = bz2 - _colsum(Wz2)
    bz3p = bz3 - _colsum(Wz3)
    bd1p = bd1 - _colsum(Wd1[:H_DIM])
    bd2p = bd2 - _colsum(Wd2)
    bd3p = bd3 - _colsum(Wd3)
    bd4p = bd4 - _colsum(Wd4)
    b_ihp = b_ih - _colsum(w_ih[:H_DIM]) - _colsum(w_ih[H_DIM:])
    bp2p = bp2 - _colsum(Wp2)
    bp3p = bp3 - _colsum(Wp3)

    Wz1e = np.concatenate([Wz1, bz1[None, :]], axis=0)       # [65, 512]
    Wx1e = np.concatenate([Wx1n, bx1n[None, :]], axis=0)     # [81, 512]

    brz = (b_ihp + b_hh)[: 2 * H_DIM]
    bin_ = b_ihp[2 * H_DIM:]
    bhn = b_hh[2 * H_DIM:]

    return {
        "y": f32(y),
        "Wx1n": Wx1n, "bx1n_a": _pc(bx1n, HC), "bx1n_b": _pc(bx1n + 1.0, HC),
        "Wx2": Wx2, "bx2_a": _pc(bx2p, HC), "bx2_b": _pc(bx2p + 1.0, HC),
        "Wx3": Wx3, "bx3_a": _pc(bx3p, HC), "bx3_b": _pc(bx3p + 1.0, HC),
        "We1t": np.ascontiguousarray(We1[:H_DIM]),
        "We1b": np.ascontiguousarray(We1[H_DIM:]),
        "beE": _pc(beE, HC),
        "We2": We2, "be2_a": _pc(be2p, HC), "be2_b": _pc(be2p + 1.0, HC),
        "We3": We3, "be3": np.ascontiguousarray(be3p.reshape(Z_DIM, 1)),
        "th3": np.ascontiguousarray((-be3p).reshape(Z_DIM, 1)),
        "Wz1e": Wz1e,
        "Wz2": Wz2, "bz2_a": _pc(bz2p, HC), "bz2_b": _pc(bz2p + 1.0, HC),
        "Wz3": Wz3, "bz3_a": _pc(bz3p, HC), "bz3_b": _pc(bz3p + 1.0, HC),
        "Wd1": Wd1, "bd1_a": _pc(bd1p, HC), "bd1_b": _pc(bd1p + 1.0, HC),
        "Wd2": Wd2, "bd2_a": _pc(bd2p, HC), "bd2_b": _pc(bd2p + 1.0, HC),
        "Wd3": Wd3, "bd3_a": _pc(bd3p, HC), "bd3_b": _pc(bd3p + 1.0, HC),
        "Wd4": Wd4, "bd4": np.ascontiguousarray(bd4p.reshape(X_DIM, 1)),
        "Wx1e": Wx1e,
        "Wih": w_ih, "Whh": w_hh,
        "brz8": _pc(brz, 8), "bin4": _pc(bin_, HC), "bhn4": _pc(bhn, HC),
        "Wp1": Wp1, "bp1_a": _pc(bp1, HC), "bp1_b": _pc(bp1 + 1.0, HC),
        "Wp2": Wp2, "bp2_a": _pc(bp2p, HC), "bp2_b": _pc(bp2p + 1.0, HC),
        "Wp3": Wp3, "bp3": np.ascontiguousarray(bp3p.reshape(Z_DIM, 1)),
    }


def _load_w(nc, pool, dram, K, M):
    """Load W [K, M] into SBUF tile [128 or K, K//128 or 1, M]."""
    tag = "w_" + dram.name
    if K > 128:
        assert K % 128 == 0
        t = pool.tile([128, K // 128, M], F32, tag=tag)
        nc.sync.dma_start(t[:], dram[:].rearrange("(c p) m -> p c m", p=128))
    else:
        t = pool.tile([K, 1, M], F32, tag=tag)
        nc.sync.dma_start(t[:, 0, :], dram[:])
    return t


def _ld_bias(nc, pool, dram, shape):
    t = pool.tile(list(shape), F32, tag="b_" + dram.name)
    nc.sync.dma_start(t[:], dram[:])
    return t


def _elu_p1(nc, mnpool, epool, psums, out, b_a, b_b, nb, extra=None, prepool=None):
    """out[:, m, :] = elu(psum_m + bias_m [+ extra_m]) + 1.

    psums: list of per-m psum APs [128, nb]. b_a/b_b: [128, C] tiles or None
    (bias already included; uses 0/+1 immediates). extra: [128, C, nb] sbuf AP
    added to the preact (enc1's E contribution).
    """
    C = len(psums)
    mn = mnpool.tile([128, C, nb], F32, tag="mn")
    if extra is not None:
        pre = prepool.tile([128, C, nb], F32, tag="pre")
        for m in range(C):
            nc.vector.scalar_tensor_tensor(pre[:, m, :], psums[m], 0.0,
                                           extra[:, m, :], op0=ALU.add, op1=ALU.add)
        for m in range(C):
            nc.vector.tensor_scalar(mn[:, m, :], pre[:, m, :], 0.0, None, op0=ALU.min)
    elif b_a is not None:
        for m in range(C):
            nc.vector.tensor_scalar(mn[:, m, :], psums[m], b_a[:, m:m + 1], 0.0,
                                    op0=ALU.add, op1=ALU.min)
    else:
        for m in range(C):
            nc.vector.tensor_scalar(mn[:, m, :], psums[m], 0.0, None, op0=ALU.min)
    e = epool.tile([128, C, nb], F32, tag="ee")
    nc.scalar.activation(e[:], mn[:], AF.Exp)
    for m in range(C):
        if extra is not None:
            nc.vector.scalar_tensor_tensor(out[:, m, :], pre[:, m, :], 1.0,
                                           e[:, m, :], op0=ALU.add, op1=ALU.max)
        elif b_b is not None:
            nc.vector.scalar_tensor_tensor(out[:, m, :], psums[m], b_b[:, m:m + 1],
                                           e[:, m, :], op0=ALU.add, op1=ALU.max)
        else:
            nc.vector.scalar_tensor_tensor(out[:, m, :], psums[m], 1.0,
                                           e[:, m, :], op0=ALU.add, op1=ALU.max)


def _build(T):
    assert T % 8 == 0
    NS = T // 8

    nc = bacc.Bacc("TRN2", target_bir_lowering=False, debug=False, num_devices=8)

    d = {}
    def din(name, shape):
        d[name] = nc.dram_tensor(name, list(shape), F32, kind="ExternalInput")
    din("y", (B, T, X_DIM))
    din("Wx1n", (X_DIM, H_DIM)); din("bx1n_a", (128, HC)); din("bx1n_b", (128, HC))
    din("Wx2", (H_DIM, H_DIM)); din("bx2_a", (128, HC)); din("bx2_b", (128, HC))
    din("Wx3", (H_DIM, H_DIM)); din("bx3_a", (128, HC)); din("bx3_b", (128, HC))
    din("We1t", (H_DIM, H_DIM)); din("We1b", (H_DIM, H_DIM)); din("beE", (128, HC))
    din("We2", (H_DIM, H_DIM)); din("be2_a", (128, HC)); din("be2_b", (128, HC))
    din("We3", (H_DIM, Z_DIM)); din("be3", (Z_DIM, 1)); din("th3", (Z_DIM, 1))
    din("Wz1e", (Z_DIM + 1, H_DIM))
    din("Wz2", (H_DIM, H_DIM)); din("bz2_a", (128, HC)); din("bz2_b", (128, HC))
    din("Wz3", (H_DIM, H_DIM)); din("bz3_a", (128, HC)); din("bz3_b", (128, HC))
    din("Wd1", (2 * H_DIM, H_DIM)); din("bd1_a", (128, HC)); din("bd1_b", (128, HC))
    din("Wd2", (H_DIM, H_DIM)); din("bd2_a", (128, HC)); din("bd2_b", (128, HC))
    din("Wd3", (H_DIM, H_DIM)); din("bd3_a", (128, HC)); din("bd3_b", (128, HC))
    din("Wd4", (H_DIM, X_DIM)); din("bd4", (X_DIM, 1))
    din("Wx1e", (X_DIM + 1, H_DIM))
    din("Wih", (2 * H_DIM, 3 * H_DIM)); din("Whh", (H_DIM, 3 * H_DIM))
    din("brz8", (128, 8)); din("bin4", (128, HC)); din("bhn4", (128, HC))
    din("Wp1", (H_DIM, H_DIM)); din("bp1_a", (128, HC)); din("bp1_b", (128, HC))
    din("Wp2", (H_DIM, H_DIM)); din("bp2_a", (128, HC)); din("bp2_b", (128, HC))
    din("Wp3", (H_DIM, Z_DIM)); din("bp3", (Z_DIM, 1))

    DEC = nc.dram_tensor("DEC", [X_DIM, NS, 8 * B], F32, kind="ExternalOutput")
    KLD = nc.dram_tensor("KLD", [Z_DIM, 1], F32, kind="ExternalOutput")

    Eflat = nc.dram_tensor("Eflat", [NS, 128, HC * 8 * B], F32)
    ENCP = nc.dram_tensor("ENCP", [Z_DIM, NS, 8 * B], F32)
    PRIP = nc.dram_tensor("PRIP", [Z_DIM, NS, 8 * B], F32)

    with tile.TileContext(nc) as tc:
        _emit_precompute(nc, tc, d, Eflat, T)
        _emit_loop(nc, tc, d, DEC, Eflat, ENCP, PRIP, NS)
        _emit_kld(nc, tc, ENCP, PRIP, KLD, NS)

    nc.compile()
    return nc


def _emit_precompute(nc, tc, d, Eflat, T):
    TCH = 120 if T % 120 == 0 else 8
    NCH = T // TCH
    ROWS = B * TCH
    assert ROWS % 512 == 0
    NSL = ROWS // 512
    TPS = 512 // B  # t's per slice = 8
    with (
        tc.tile_pool(name="pw", bufs=1) as pw,
        tc.tile_pool(name="pa", bufs=2) as pa,
        tc.tile_pool(name="pyt", bufs=2) as pyt,
        tc.tile_pool(name="pps", bufs=6, space="PSUM") as pps,
        tc.tile_pool(name="ppt", bufs=2, space="PSUM") as ppt,
    ):
        ident = pw.tile([128, 128], F32, tag="ident")
        make_identity(nc, ident[:])
        Wx1n_t = _load_w(nc, pw, d["Wx1n"], X_DIM, H_DIM)
        Wx2_t = _load_w(nc, pw, d["Wx2"], H_DIM, H_DIM)
        Wx3_t = _load_w(nc, pw, d["Wx3"], H_DIM, H_DIM)
        We1t_t = _load_w(nc, pw, d["We1t"], H_DIM, H_DIM)
        bb = {}
        for nm in ["bx1n_a", "bx1n_b", "bx2_a", "bx2_b", "bx3_a", "bx3_b", "beE"]:
            bb[nm] = _ld_bias(nc, pw, d[nm], (128, HC))

        for ch in range(NCH):
            yT = pyt.tile([X_DIM, ROWS], F32, tag="yT")
            for b in range(B):
                yblk = pa.tile([TCH, X_DIM], F32, tag="yblk")
                nc.sync.dma_start(yblk[:], d["y"][b, ch * TCH:(ch + 1) * TCH, :])
                tp = ppt.tile([X_DIM, TCH], F32, tag="tp")
                nc.tensor.transpose(tp[:], yblk[:], ident[:TCH, :TCH])
                nc.vector.tensor_copy(yT[:, b * TCH:(b + 1) * TCH], tp[:])

            yTv = yT[:].rearrange("p (b t) -> p t b", b=B)
            for s in range(NSL):
                rhs1 = yTv[:, s * TPS:(s + 1) * TPS, :]

                def layer(W_t, rhs, KC, b_a, b_b, out_tag, single_k=False):
                    psums = []
                    for m in range(HC):
                        pm = pps.tile([128, 512], F32, tag="ps")
                        if single_k:
                            nc.tensor.matmul(pm[:], W_t[:, 0, ts(m, 128)], rhs,
                                             start=True, stop=True)
                        else:
                            for c in range(HC):
                                nc.tensor.matmul(pm[:], W_t[:, c, ts(m, 128)],
                                                 rhs[:, c, :], start=(c == 0),
                                                 stop=(c == HC - 1))
                        psums.append(pm[:])
                    out = pa.tile([128, HC, 512], F32, tag=out_tag)
                    _elu_p1(nc, pa, pa, psums, out, b_a, b_b, 512)
                    return out

                a1 = layer(Wx1n_t, rhs1, 1, bb["bx1n_a"], bb["bx1n_b"], "a1", single_k=True)
                a2 = layer(Wx2_t, a1, HC, bb["bx2_a"], bb["bx2_b"], "a2")
                a3 = layer(Wx3_t, a2, HC, bb["bx3_a"], bb["bx3_b"], "a1")
                eo = pa.tile([128, HC, 512], F32, tag="a2")
                for m in range(HC):
                    pm = pps.tile([128, 512], F32, tag="ps")
                    for c in range(HC):
                        nc.tensor.matmul(pm[:], We1t_t[:, c, ts(m, 128)], a3[:, c, :],
                                         start=(c == 0), stop=(c == HC - 1))
                    nc.vector.tensor_scalar_add(eo[:, m, :], pm[:], bb["beE"][:, m:m + 1])
                gs = (ch * TCH) // TPS + s
                nc.sync.dma_start(
                    Eflat[gs, :, :].rearrange("p (c x) -> p c x", c=HC), eo[:])


def _emit_loop(nc, tc, d, DEC, Eflat, ENCP, PRIP, NS):
    with (
        tc.tile_pool(name="lw", bufs=1) as lw,
        tc.tile_pool(name="st", bufs=1) as st,
        tc.tile_pool(name="io", bufs=1) as io,
        tc.tile_pool(name="e8", bufs=1) as e8p,
        tc.tile_pool(name="ac", bufs=2) as ac,
        tc.tile_pool(name="sm", bufs=1) as sm,
        tc.tile_pool(name="mne", bufs=1) as mne,
        tc.tile_pool(name="lps", bufs=6, space="PSUM") as lps,
        tc.tile_pool(name="gps", bufs=2, space="PSUM") as gps,
    ):
        W = {}
        W["e1b"] = _load_w(nc, lw, d["We1b"], H_DIM, H_DIM)
        W["e2"] = _load_w(nc, lw, d["We2"], H_DIM, H_DIM)
        W["e3"] = _load_w(nc, lw, d["We3"], H_DIM, Z_DIM)
        W["z1"] = _load_w(nc, lw, d["Wz1e"], Z_DIM + 1, H_DIM)
        W["z2"] = _load_w(nc, lw, d["Wz2"], H_DIM, H_DIM)
        W["z3"] = _load_w(nc, lw, d["Wz3"], H_DIM, H_DIM)
        W["d1"] = _load_w(nc, lw, d["Wd1"], 2 * H_DIM, H_DIM)
        W["d2"] = _load_w(nc, lw, d["Wd2"], H_DIM, H_DIM)
        W["d3"] = _load_w(nc, lw, d["Wd3"], H_DIM, H_DIM)
        W["d4"] = _load_w(nc, lw, d["Wd4"], H_DIM, X_DIM)
        W["x1"] = _load_w(nc, lw, d["Wx1e"], X_DIM + 1, H_DIM)
        W["x2"] = _load_w(nc, lw, d["Wx2"], H_DIM, H_DIM)
        W["x3"] = _load_w(nc, lw, d["Wx3"], H_DIM, H_DIM)
        W["ih"] = _load_w(nc, lw, d["Wih"], 2 * H_DIM, 3 * H_DIM)
        W["hh"] = _load_w(nc, lw, d["Whh"], H_DIM, 3 * H_DIM)
        W["p1"] = _load_w(nc, lw, d["Wp1"], H_DIM, H_DIM)
        W["p2"] = _load_w(nc, lw, d["Wp2"], H_DIM, H_DIM)
        W["p3"] = _load_w(nc, lw, d["Wp3"], H_DIM, Z_DIM)

        bias = {}
        for nm in ["bx2", "bx3", "be2", "bz2", "bz3", "bd1", "bd2", "bd3",
                   "bp1", "bp2"]:
            bias[nm + "_a"] = _ld_bias(nc, lw, d[nm + "_a"], (128, HC))
            bias[nm + "_b"] = _ld_bias(nc, lw, d[nm + "_b"], (128, HC))
        bias["be3"] = _ld_bias(nc, lw, d["be3"], (Z_DIM, 1))
        bias["th3"] = _ld_bias(nc, lw, d["th3"], (Z_DIM, 1))
        bias["bd4"] = _ld_bias(nc, lw, d["bd4"], (X_DIM, 1))
        bias["bp3"] = _ld_bias(nc, lw, d["bp3"], (Z_DIM, 1))
        brz_t = _ld_bias(nc, lw, d["brz8"], (128, 8))
        bin_t = _ld_bias(nc, lw, d["bin4"], (128, HC))
        bhn_t = _ld_bias(nc, lw, d["bhn4"], (128, HC))

        h = st.tile([128, HC, B], F32, tag="h")
        nc.vector.memset(h[:], 0.0)

        with tc.For_i(0, NS, 1, hint_engines=tuple(mybir.ALL_ENGINES)) as si:
            EP8 = io.tile([128, 8, B], F32, tag="EP8")  # enc preacts [:64], prior [64:]
            DEC8 = io.tile([X_DIM, 8, B], F32, tag="DEC8")
            E8 = e8p.tile([128, HC, 8, B], F32, tag="E8")
            nc.sync.dma_start(E8[:].rearrange("p c t b -> p (c t b)"),
                              Eflat[ds(si, 1), :, :].rearrange("o p x -> p (o x)"))
            for tl in range(8):
                _emit_step(nc, ac, sm, mne, lps, gps, h, E8, EP8, DEC8, tl,
                           W, bias, brz_t, bin_t, bhn_t)
            nc.sync.dma_start(
                DEC[:, ds(si, 1), :].rearrange("p o x -> p (o x)"),
                DEC8[:].rearrange("p t b -> p (t b)"))

            nc.sync.dma_start(
                ENCP[:, ds(si, 1), :].rearrange("p o x -> p (o x)"),
                EP8[:Z_DIM, :, :].rearrange("p t b -> p (t b)"))
            nc.sync.dma_start(
                PRIP[:, ds(si, 1), :].rearrange("p o x -> p (o x)"),
                EP8[Z_DIM:, :, :].rearrange("p t b -> p (t b)"))


def _mm(nc, psum_ap, W_t, rhs, KC, mslice, start=True, stop=True):
    for c in range(KC):
        nc.tensor.matmul(psum_ap, W_t[:, c, mslice], rhs[:, c, :],
                         start=(start and c == 0), stop=(stop and c == KC - 1))


def _emit_step(nc, ac, sm, mne, lps, gps, h, E8, EP8, DEC8, tl,
               W, bias, brz_t, bin_t, bhn_t):
    def hc_layer(W_t, rhs, b_a, b_b, out_tag, single_k=False, extra=None,
                 dual=None):
        psums = []
        for m in range(HC):
            pm = lps.tile([128, B], F32, tag="ps")
            if single_k:
                nc.tensor.matmul(pm[:], W_t[:, 0, ts(m, 128)], rhs, start=True, stop=True)
            else:
                for c in range(HC):
                    nc.tensor.matmul(pm[:], W_t[:, c, ts(m, 128)], rhs[:, c, :],
                                     start=(c == 0), stop=(dual is None and c == HC - 1))
                if dual is not None:
                    W2_t, rhs2, koff = dual
                    for c in range(HC):
                        nc.tensor.matmul(pm[:], W2_t[:, koff + c, ts(m, 128)],
                                         rhs2[:, c, :], start=False, stop=(c == HC - 1))
            psums.append(pm[:])
        out = ac.tile([128, HC, B], F32, tag=out_tag)
        _elu_p1(nc, mne, mne, psums, out, b_a, b_b, B, extra=extra, prepool=mne)
        return out

    # ---- prior (reads h before update; off the critical path) ----
    p1a = hc_layer(W["p1"], h, bias["bp1_a"], bias["bp1_b"], "pact")
    p2a = hc_layer(W["p2"], p1a, bias["bp2_a"], bias["bp2_b"], "pact")
    pr3 = lps.tile([Z_DIM, B], F32, tag="ps")
    _mm(nc, pr3[:], W["p3"], p2a, HC, slice(0, Z_DIM))
    nc.vector.tensor_scalar_add(EP8[Z_DIM:, tl, :], pr3[:], bias["bp3"][:])

    # ---- enc ----
    e1a = hc_layer(W["e1b"], h, None, None, "act", extra=E8[:, :, tl, :])
    e2a = hc_layer(W["e2"], e1a, bias["be2_a"], bias["be2_b"], "act")
    p3 = lps.tile([Z_DIM, B], F32, tag="ps")
    _mm(nc, p3[:], W["e3"], e2a, HC, slice(0, Z_DIM))
    zx = sm.tile([Z_DIM + 1, B], F32, tag="zx")
    nc.vector.tensor_scalar(zx[:Z_DIM, :], p3[:], bias["th3"][:], None, op0=ALU.is_ge)
    nc.vector.memset(zx[Z_DIM:, :], 1.0)
    nc.vector.tensor_scalar_add(EP8[:Z_DIM, tl, :], p3[:], bias["be3"][:])

    # ---- phi_z ----
    z1a = hc_layer(W["z1"], zx[:], None, None, "act", single_k=True)
    z2a = hc_layer(W["z2"], z1a, bias["bz2_a"], bias["bz2_b"], "act")
    z3a = hc_layer(W["z3"], z2a, bias["bz3_a"], bias["bz3_b"], "z3a")

    # ---- dec ----
    d1a = hc_layer(W["d1"], z3a, bias["bd1_a"], bias["bd1_b"], "act",
                   dual=(W["d1"], h, HC))
    d2a = hc_layer(W["d2"], d1a, bias["bd2_a"], bias["bd2_b"], "act")
    d3a = hc_layer(W["d3"], d2a, bias["bd3_a"], bias["bd3_b"], "act")
    p4 = lps.tile([X_DIM, B], F32, tag="ps")
    _mm(nc, p4[:], W["d4"], d3a, HC, slice(0, X_DIM))
    dx = sm.tile([96, B], F32, tag="dx")
    nc.vector.memset(dx[64:96, :], 1.0)
    nc.vector.tensor_scalar_add(dx[:X_DIM, :], p4[:], bias["bd4"][:])
    nc.vector.tensor_copy(DEC8[:, tl, :], dx[:X_DIM, :])

    # ---- phi_x_gen ----
    x1a = hc_layer(W["x1"], dx[:X_DIM + 1, :], None, None, "act", single_k=True)
    x2a = hc_layer(W["x2"], x1a, bias["bx2_a"], bias["bx2_b"], "act")
    x3a = hc_layer(W["x3"], x2a, bias["bx3_a"], bias["bx3_b"], "x3a")

    # ---- GRU matmuls ----
    gi = gps.tile([128, 8, B], F32, tag="gi")
    gn = lps.tile([128, HC, B], F32, tag="ps")
    hn = lps.tile([128, HC, B], F32, tag="ps")
    for m in range(8):
        for c in range(HC):
            nc.tensor.matmul(gi[:, m, :], W["ih"][:, c, ts(m, 128)], x3a[:, c, :],
                             start=(c == 0), stop=False)
        for c in range(HC):
            nc.tensor.matmul(gi[:, m, :], W["ih"][:, HC + c, ts(m, 128)], z3a[:, c, :],
                             start=False, stop=False)
        for c in range(HC):
            nc.tensor.matmul(gi[:, m, :], W["hh"][:, c, ts(m, 128)], h[:, c, :],
                             start=False, stop=(c == HC - 1))
    for j in range(HC):
        for c in range(HC):
            nc.tensor.matmul(gn[:, j, :], W["ih"][:, c, ts(8 + j, 128)], x3a[:, c, :],
                             start=(c == 0), stop=False)
        for c in range(HC):
            nc.tensor.matmul(gn[:, j, :], W["ih"][:, HC + c, ts(8 + j, 128)], z3a[:, c, :],
                             start=False, stop=(c == HC - 1))
        for c in range(HC):
            nc.tensor.matmul(hn[:, j, :], W["hh"][:, c, ts(8 + j, 128)], h[:, c, :],
                             start=(c == 0), stop=(c == HC - 1))

    # ---- gates: r and z via sigmoid = 1/(1+exp(-pre)) ----
    def gate_rz(lo, out_tag):
        pre = sm.tile([128, HC, B], F32, tag="rzp")
        brz_v = brz_t[:, lo:lo + HC, None].broadcast_to([128, HC, B])
        nc.vector.tensor_tensor(pre[:], gi[:, lo:lo + HC, :], brz_v, op=ALU.add)
        nc.scalar.activation(pre[:], pre[:], AF.Exp, scale=-1.0)
        nc.vector.tensor_scalar_add(pre[:], pre[:], 1.0)
        out = sm.tile([128, HC, B], F32, tag=out_tag)
        nc.vector.reciprocal(out[:], pre[:])
        return out

    r = gate_rz(0, "r")
    z = gate_rz(HC, "z")

    # n = tanh(gn + bin + r * (hn + bhn)); tanh(x) = 2/(1+exp(-2x)) - 1
    t1 = sm.tile([128, HC, B], F32, tag="gA")
    bhn_v = bhn_t[:, :, None].broadcast_to([128, HC, B])
    nc.vector.tensor_tensor(t1[:], hn[:], bhn_v, op=ALU.add)
    nc.vector.tensor_tensor(t1[:], t1[:], r[:], op=ALU.mult)
    t2 = sm.tile([128, HC, B], F32, tag="gB")
    for m in range(HC):
        nc.vector.scalar_tensor_tensor(t2[:, m, :], gn[:, m, :], 0.0, t1[:, m, :],
                                       op0=ALU.add, op1=ALU.add)
    bin_v = bin_t[:, :, None].broadcast_to([128, HC, B])
    nc.vector.tensor_tensor(t2[:], t2[:], bin_v, op=ALU.add)
    en = sm.tile([128, HC, B], F32, tag="gA")
    nc.scalar.activation(en[:], t2[:], AF.Exp, scale=-2.0)
    nc.vector.tensor_scalar_add(en[:], en[:], 1.0)
    nt = sm.tile([128, HC, B], F32, tag="nt")
    nc.vector.reciprocal(nt[:], en[:])
    nc.vector.tensor_scalar(nt[:], nt[:], 2.0, -1.0, op0=ALU.mult, op1=ALU.add)

    # h = n + z*(h - n)
    dh = sm.tile([128, HC, B], F32, tag="gB")
    nc.vector.tensor_tensor(dh[:], h[:], nt[:], op=ALU.subtract)
    nc.vector.tensor_tensor(dh[:], dh[:], z[:], op=ALU.mult)
    nc.vector.tensor_tensor(h[:], nt[:], dh[:], op=ALU.add)


def _emit_kld(nc, tc, ENCP, PRIP, KLD, NS):
    SC = max(dv for dv in range(1, min(NS, 5) + 1) if NS % dv == 0)
    KCH = NS // SC
    NB = SC * 8 * B
    with (
        tc.tile_pool(name="ka", bufs=1) as ka,
        tc.tile_pool(name="kc", bufs=1) as kc,
    ):
        acc = kc.tile([Z_DIM, 1], F32, tag="acc")
        nc.vector.memset(acc[:], 0.0)

        def sig(u, tg):
            e = ka.tile([Z_DIM, NB], F32, tag="sge")
            nc.scalar.activation(e[:], u[:], AF.Exp, scale=-1.0)
            nc.vector.tensor_scalar_add(e[:], e[:], 1.0)
            o = ka.tile([Z_DIM, NB], F32, tag=tg)
            nc.vector.reciprocal(o[:], e[:])
            return o

        def lnclip(x, tg):
            c = ka.tile([Z_DIM, NB], F32, tag="lncs")
            nc.vector.tensor_scalar_max(c[:], x[:], 0.001)
            o = ka.tile([Z_DIM, NB], F32, tag=tg)
            nc.scalar.activation(o[:], c[:], AF.Ln)
            return o

        for k in range(KCH):
            up = ka.tile([Z_DIM, NB], F32, tag="up")
            nc.sync.dma_start(up[:], ENCP[:, k * SC:(k + 1) * SC, :]
                              .rearrange("p s x -> p (s x)"))
            vp = ka.tile([Z_DIM, NB], F32, tag="vp")
            nc.sync.dma_start(vp[:], PRIP[:, k * SC:(k + 1) * SC, :]
                              .rearrange("p s x -> p (s x)"))
            enc = sig(up, "enc")
            pri = sig(vp, "pri")
            ome = ka.tile([Z_DIM, NB], F32, tag="om")
            nc.vector.tensor_scalar(ome[:], enc[:], -1.0, 1.0, op0=ALU.mult, op1=ALU.add)
            omp = ka.tile([Z_DIM, NB], F32, tag="om2")
            nc.vector.tensor_scalar(omp[:], pri[:], -1.0, 1.0, op0=ALU.mult, op1=ALU.add)
            l1 = lnclip(enc, "l1")
            l2 = lnclip(pri, "l2")
            l3 = lnclip(ome, "l3")
            l4 = lnclip(omp, "l4")
            nc.vector.tensor_tensor(l1[:], l1[:], l2[:], op=ALU.subtract)
            nc.vector.tensor_tensor(l3[:], l3[:], l4[:], op=ALU.subtract)
            nc.vector.tensor_tensor(l1[:], enc[:], l1[:], op=ALU.mult)
            nc.vector.tensor_tensor(l3[:], ome[:], l3[:], op=ALU.mult)
            nc.vector.tensor_tensor(l1[:], l1[:], l3[:], op=ALU.add)
            part = ka.tile([Z_DIM, 1], F32, tag="pt")
            nc.vector.tensor_reduce(part[:], l1[:], axis=mybir.AxisListType.X, op=ALU.add)
            nc.vector.tensor_tensor(acc[:], acc[:], part[:], op=ALU.add)
        nc.sync.dma_start(KLD[:], acc[:])


def run_resident(nc, prepped, n_iters=2):
    """Execute with device-resident inputs; returns (results_core0, best_exec_s).

    Mirrors bass2jax.run_bass_via_pjrt's multi-core path but keeps the jitted
    callable and input arrays on device so repeat calls measure device
    execution (+ small dispatch) rather than host->device transfer.
    """
    import time
    import jax
    import jax.numpy as jnp
    from jax.sharding import Mesh, PartitionSpec
    from jax.experimental.shard_map import shard_map
    from concourse import bass2jax as b2j
    from concourse import mybir as mb

    b2j.install_neuronx_cc_hook()
    n_cores = 8
    partition_name = nc.partition_id_tensor.name if nc.partition_id_tensor else None
    in_names, out_names, out_avals = [], [], []
    for alloc in nc.m.functions[0].allocations:
        if not isinstance(alloc, mb.MemoryLocationSet):
            continue
        name = alloc.memorylocations[0].name
        if alloc.kind == "ExternalInput":
            if name != partition_name:
                in_names.append(name)
        elif alloc.kind == "ExternalOutput":
            out_names.append(name)
            out_avals.append(jax.core.ShapedArray(tuple(alloc.tensor_shape),
                                                  mb.dt.np(alloc.dtype)))
    n_params = len(in_names)
    all_in_names = list(in_names) + list(out_names)
    if partition_name is not None:
        all_in_names.append(partition_name)
    donate = tuple(range(n_params, n_params + len(out_names)))

    def _body(*args):
        operands = list(args)
        if partition_name is not None:
            operands.append(b2j.partition_id_tensor())
        return tuple(b2j._bass_exec_p.bind(
            *operands, out_avals=tuple(out_avals), in_names=tuple(all_in_names),
            out_names=tuple(out_names), lowering_input_output_aliases=(),
            sim_require_finite=True, sim_require_nnan=True, nc=nc))

    devices = jax.devices()[:n_cores]
    mesh = Mesh(np.asarray(devices), ("core",))
    nio = n_params + len(out_names)
    sharded = jax.jit(
        shard_map(_body, mesh=mesh, in_specs=(PartitionSpec("core"),) * nio,
                  out_specs=(PartitionSpec("core"),) * len(out_names),
                  check_rep=False),
        donate_argnums=donate, keep_unused=True)

    from jax.sharding import NamedSharding
    shard = NamedSharding(mesh, PartitionSpec("core"))
    concat_in = [jax.device_put(
        np.concatenate([np.asarray(prepped[nm])] * n_cores, axis=0), shard)
        for nm in in_names]

    def zeros():
        return [jax.device_put(
            np.zeros((n_cores * a.shape[0], *a.shape[1:]), a.dtype), shard)
            for a in out_avals]

    # warm-up (compiles)
    out = sharded(*concat_in, *zeros())
    jax.block_until_ready(out)
    best = None
    for _ in range(n_iters):
        z = zeros()
        jax.block_until_ready(z)
        t0 = time.time()
        out = sharded(*concat_in, *z)
        jax.block_until_ready(out)
        dt = time.time() - t0
        best = dt if best is None or dt < best else best
    res = {name: np.asarray(out[i]).reshape(n_cores, *out_avals[i].shape)[0]
           for i, name in enumerate(out_names)}
    return res, best


def kernel(**inputs):
    T = inputs["y"].shape[1]
    prepped = _prep_inputs(
        inputs["y"], inputs["mean_mel"], inputs["std_mel"], inputs["phi_x_params"],
        inputs["phi_z_params"], inputs["enc_params"], inputs["prior_params"],
        inputs["dec_params"], inputs["gru_params"])
    if T not in _CACHE:
        _CACHE[T] = _build(T)
    nc = _CACHE[T]
    res = run_bass_kernel_spmd(nc, [prepped] * 8, list(range(8))).results
    dec = res[0]["DEC"].reshape(X_DIM, T, B).transpose(2, 1, 0)
    kld = np.float32(res[0]["KLD"].sum() / (T * B))
    return np.ascontiguousarray(dec), kld
